# revision 55
# baseline (speedup 1.0000x reference)
"""Distributed Trainium2 Bass kernel for BitNet-style attention block.

Sharding: sequence-parallel projections + (batch x kv-head) parallel attention,
stitched with per-batch AllToAll collectives (split so comm overlaps compute).

Per core (core i):
  A. RMSNorm + per-token absmax quantization of its 512-token chunk.
  B. qkv projection as exact integer bf16 matmul against host-prequantized
     ternary weights, dequant, RoPE on q/k, scatter into per-batch A2A bufs.
  C. AllToAll #1 (k/v/qA/qB fired as their data completes) -> core i holds
     full-sequence q/k/v for kv-head i of each batch; causal attention with
     transposed scores, exp on ACT, and a TRANSPOSED attention*V (stationary
     [v|ones], moving probabilities, 512-wide) that emits out^T [d, qi] plus
     rowsums; normalization via gpsimd partition-broadcast of 1/rowsum.
  D. AllToAll #2 ships out^T (hidden-major rows) -> core i holds x^T
     [2048 hidden, 256 tok] per batch; per-token quantization via gpsimd
     partition all-reduce absmax, integer matmul with ternary output weights
     (no PE transposes needed - x^T rows are already the contract dim).
"""
import sys
sys.path.insert(0, "/opt/trn_rl_repo")
import numpy as np
import ml_dtypes
import concourse.bass as bass
import concourse.tile as tile
from concourse import bacc, mybir
from concourse import bass_utils
from concourse import bass_isa
from concourse.masks import make_identity

f32 = mybir.dt.float32
bf16 = mybir.dt.bfloat16
f8 = mybir.dt.float8e4
FT = mybir.ActivationFunctionType
ALU = mybir.AluOpType

B, S, H = 2, 2048, 2048
NH, NKV, HD = 32, 8, 64
G = NH // NKV                    # 4
QKV_O = (NH + 2 * NKV) * HD      # 3072
EPS = 1e-5
THETA = 10000.0
C = 8
SC = S // C                      # 256 positions per core
TOK = B * SC                     # 512 token rows per core
MAGIC = float(1.5 * 2.0 ** 23)   # RNE integer rounding for |v| < 2^22
NT = TOK // 128                  # 4 token tiles
NHT = H // 128                   # 16 h-tiles
NKT = S // 128                   # 16 kj tiles

# a2a1 split into four column-group collectives fired as their data completes:
#   k: per-batch [k 64] -> [8, 128, 128]  (dup built locally via 2 transpose loads)
#   v: per-batch [v 64] -> [8, 128, 128]
#   qA/qB: per-batch head-pair [2 heads = 128] -> [8, 256, 256] each
# q heads are permuted host-side: col hp*1024 + dest*128 + hh*64 + d
# a2a2 is transposed: rows = dest*256 + head*64 + d, cols = dest's 256 tokens.


def _dap(t_ap, extra, dims):
    return bass.AP(tensor=t_ap.tensor, offset=t_ap.offset + extra, ap=[list(d) for d in dims])


def build_nc():
    nc = bacc.Bacc("TRN2", target_bir_lowering=False, debug=False, num_devices=C)

    x_in = nc.dram_tensor("x", [TOK, H], f32, kind="ExternalInput")
    wn_in = nc.dram_tensor("wn", [1, H], f32, kind="ExternalInput")
    # contiguous stripes: row ((ng*16+j)*128 + h_local), 512 cols each
    wq1t_in = nc.dram_tensor("wq1t", [(QKV_O // 512) * NHT * 128, 512], f8, kind="ExternalInput")
    wq2t_in = nc.dram_tensor("wq2t", [(H // 512) * NHT * 128, 512], f8, kind="ExternalInput")
    cos_in = nc.dram_tensor("cosb", [SC, 8 * 32], f32, kind="ExternalInput")
    sin_in = nc.dram_tensor("sinb", [SC, 8 * 32], f32, kind="ExternalInput")
    tri_in = nc.dram_tensor("trimask", [128, 128], bf16, kind="ExternalInput")
    sw1_in = nc.dram_tensor("sw1", [1, 1], f32, kind="ExternalInput")
    sw2_in = nc.dram_tensor("sw2", [1, 1], f32, kind="ExternalInput")
    out_ext = nc.dram_tensor("out", [TOK, H], f32, kind="ExternalOutput")

    X = x_in.ap()
    WQ1 = wq1t_in.ap()
    WQ2 = wq2t_in.ap()
    OUT = out_ext.ap()

    with tile.TileContext(nc) as tc:
        from contextlib import ExitStack
        with ExitStack() as top:
            dram = top.enter_context(tc.tile_pool(name="dram", bufs=1, space="DRAM"))
            const = top.enter_context(tc.tile_pool(name="const", bufs=1))
            smalls = top.enter_context(tc.tile_pool(name="smalls", bufs=1))
            psA = top.enter_context(tc.tile_pool(name="psA", bufs=4, space="PSUM"))
            psS = top.enter_context(tc.tile_pool(name="psS", bufs=2, space="PSUM"))

            # ---------------- DRAM scratch ----------------
            aq_i = [dram.tile([C * SC, 256], bf16, name=f"aq_i{hp}") for hp in range(2)]
            aq_o = [dram.tile([C * SC, 256], bf16, name=f"aq_o{hp}") for hp in range(2)]
            ak_i = dram.tile([C * SC, 128], bf16, name="ak_i")
            ak_o = dram.tile([C * SC, 128], bf16, name="ak_o")
            av_i = dram.tile([C * SC, 128], bf16, name="av_i")
            av_o = dram.tile([C * SC, 128], bf16, name="av_o")
            a2i = [dram.tile([C * SC, G * HD], bf16, name=f"a2i_{b}") for b in range(B)]
            a2o = [dram.tile([C * SC, G * HD], bf16, name=f"a2o_{b}") for b in range(B)]
            d2dr = dram.tile([B, SC], f32, name="d2dr")

            # ---------------- constants ----------------
            wnorm_b = const.tile([128, H], f32)
            wnr = const.tile([1, H], f32)
            nc.sync.dma_start(out=wnr[0:1, :], in_=wn_in.ap()[0:1, :])
            nc.gpsimd.partition_broadcast(wnorm_b[:, :], wnr[0:1, :], channels=128)
            trim = const.tile([128, 128], bf16)
            nc.sync.dma_start(out=trim[:], in_=tri_in.ap()[:, :])
            sw1b = const.tile([128, 1], f32)
            nc.sync.dma_start(out=sw1b[:], in_=_dap(sw1_in.ap(), 0, [[0, 128], [1, 1]]))
            sw2b = const.tile([128, 1], f32)
            nc.sync.dma_start(out=sw2b[:], in_=_dap(sw2_in.ap(), 0, [[0, 128], [1, 1]]))
            epsb = const.tile([128, 1], f32)
            nc.vector.memset(epsb[:], EPS)
            magicb = const.tile([128, 1], f32)
            nc.vector.memset(magicb[:], MAGIC)
            ident = const.tile([128, 128], bf16)
            make_identity(nc, ident[:])

            d1s = [smalls.tile([128, 1], f32, name=f"d1_{m}") for m in range(NT)]

            xqT_pool = top.enter_context(tc.tile_pool(name="xqT", bufs=NHT))
            pQT = top.enter_context(tc.tile_pool(name="pQT", bufs=4))
            pKT = top.enter_context(tc.tile_pool(name="pKT", bufs=3))
            pVA = top.enter_context(tc.tile_pool(name="pVA", bufs=2 * NKT))

            # ================= Stage A: RMSNorm + quantize =================
            with ExitStack() as sa:
                pA = sa.enter_context(tc.tile_pool(name="pA", bufs=2))
                pXQ = sa.enter_context(tc.tile_pool(name="pXQ", bufs=NT))
                pSc = sa.enter_context(tc.tile_pool(name="pASc", bufs=4))
                xqms = []
                for m in range(NT):
                    xa = pA.tile([128, H], f32, tag="xa")
                    for xc in range(8):
                        nc.sync.dma_start(out=xa[:, xc * 256:(xc + 1) * 256],
                                          in_=X[m * 128:(m + 1) * 128, xc * 256:(xc + 1) * 256])
                    sq = pA.tile([128, H], f32, tag="sq")
                    ssq = pSc.tile([128, 1], f32, tag="ssq")
                    nc.scalar.activation(out=sq[:], in_=xa[:], func=FT.Square, accum_out=ssq[:])
                    xw = pA.tile([128, H], f32, tag="xw")
                    nc.vector.tensor_tensor(xw[:], xa[:], wnorm_b[:], ALU.mult)
                    std = pSc.tile([128, 1], f32, tag="std")
                    nc.scalar.activation(out=std[:], in_=ssq[:], func=FT.Sqrt,
                                         bias=epsb[:], scale=1.0 / H)
                    rstd = pSc.tile([128, 1], f32, tag="rstd")
                    nc.vector.reciprocal(rstd[:], std[:])
                    mx = pSc.tile([128, 1], f32, tag="mx")
                    nc.vector.tensor_reduce(mx[:], xw[:], mybir.AxisListType.X, ALU.max,
                                            apply_absolute_value=True)
                    mp = pSc.tile([128, 1], f32, tag="mp")
                    nc.vector.tensor_scalar(mp[:], mx[:], rstd[:], 1e-5, ALU.mult, ALU.max)
                    nc.vector.tensor_tensor(d1s[m][:], mp[:], sw1b[:], ALU.mult)
                    rmp = pSc.tile([128, 1], f32, tag="rmp")
                    nc.vector.reciprocal(rmp[:], mp[:])
                    csc = pSc.tile([128, 1], f32, tag="csc")
                    nc.vector.tensor_scalar(csc[:], rmp[:], rstd[:], 127.0, ALU.mult, ALU.mult)
                    t1 = pA.tile([128, H], f32, tag="t1")
                    nc.scalar.activation(out=t1[:], in_=xw[:], func=FT.Identity,
                                         scale=csc[:], bias=magicb[:])
                    xqm = pXQ.tile([128, H], bf16, tag="xqm", name=f"xqm_{m}")
                    nc.vector.tensor_scalar(xqm[:], t1[:], MAGIC, None, ALU.subtract)
                    xqms.append(xqm)

                # transposed activations via PE (keeps the DMA queues free)
                xqT = []
                for j in range(NHT):
                    t = xqT_pool.tile([128, TOK], bf16, name=f"xqT_{j}", tag="xqT")
                    xqT.append(t)
                for m in range(NT):
                    for j in range(NHT):
                        tp = psS.tile([128, 128], bf16, tag="st", name=f"tp_{m}_{j}")
                        nc.tensor.transpose(tp[:], xqms[m][:, j * 128:(j + 1) * 128], ident[:])
                        nc.vector.tensor_copy(xqT[j][:, m * 128:(m + 1) * 128], tp[:])

            # ================= Stage B: qkv matmul + RoPE + scatter ========
            with ExitStack() as sb:
                pW = sb.enter_context(tc.tile_pool(name="pW", bufs=40))
                pQC = sb.enter_context(tc.tile_pool(name="pQC", bufs=7))
                pRT = sb.enter_context(tc.tile_pool(name="pRT", bufs=3))
                pSend = sb.enter_context(tc.tile_pool(name="pSend", bufs=NT))
                pCos = sb.enter_context(tc.tile_pool(name="pCos", bufs=1))

                cosr = []
                sinr = []
                for par in range(2):
                    ct = pCos.tile([128, 8 * 32], f32, name=f"cosr_{par}")
                    nc.sync.dma_start(out=ct[:], in_=cos_in.ap()[par * 128:(par + 1) * 128, :])
                    st_ = pCos.tile([128, 8 * 32], f32, name=f"sinr_{par}")
                    nc.sync.dma_start(out=st_[:], in_=sin_in.ap()[par * 128:(par + 1) * 128, :])
                    cosr.append(ct)
                    sinr.append(st_)

                sends = [pSend.tile([128, QKV_O], bf16, name=f"sends_{m}", tag="sends")
                         for m in range(NT)]

                qTs = [[None] * 2 for _ in range(B)]
                KBs = [None] * B
                vas = [[None] * NKT for _ in range(B)]

                def load_kb(eng):
                    # KBoth rows 0:64 = k(b0), rows 64:128 = k(b1); per-batch
                    # dup tiles give each batch both PE row halves.
                    KBoth = pKT.tile([128, S], bf16, name="KBoth", tag="kT")
                    KB0d = pKT.tile([128, S], bf16, name="KB0d", tag="kT")
                    KB1d = pKT.tile([128, S], bf16, name="KB1d", tag="kT")
                    for c4 in range(4):
                        cs = slice(c4 * 512, (c4 + 1) * 512)
                        eng.dma_start(out=KBoth[:, cs],
                                      in_=ak_o[c4 * 512:(c4 + 1) * 512, :],
                                      transpose=True)
                        nc.gpsimd.dma_start(out=KB0d[64:128, cs], in_=KBoth[0:64, cs])
                        nc.gpsimd.dma_start(out=KB1d[0:64, cs], in_=KBoth[64:128, cs])
                    KBs[0] = (KBoth, KB0d)
                    KBs[1] = (KB1d, KBoth)

                def load_qt(hp, b, eng):
                    t = pQT.tile([128, S], bf16, name=f"qT_{b}_{hp}", tag="qT")
                    for c4 in range(4):
                        eng.dma_start(
                            out=t[:, c4 * 512:(c4 + 1) * 512],
                            in_=aq_o[hp][c4 * 512:(c4 + 1) * 512,
                                         b * 128:(b + 1) * 128],
                            transpose=True)
                    qTs[b][hp] = t

                def load_vas(eng):
                    for b in range(B):
                        for kt in range(NKT):
                            t = pVA.tile([128, 65], bf16, name=f"va_{b}_{kt}", tag="va")
                            eng.dma_start(
                                out=t[:, 0:64],
                                in_=av_o[kt * 128:(kt + 1) * 128, b * 64:(b + 1) * 64])
                            nc.vector.memset(t[:, 64:65], 1.0)
                            vas[b][kt] = t

                for ng in (4, 0, 1, 5, 2, 3):
                    psq = [psA.tile([128, 512], f32, tag="acc", name=f"qkvp_{ng}_{m}")
                           for m in range(NT)]
                    for j in range(NHT):
                        wt = pW.tile([128, 512], f8, tag="w1")
                        r0 = (ng * NHT + j) * 128
                        nc.sync.dma_start(out=wt[:], in_=WQ1[r0:r0 + 128, :])
                        for m in range(NT):
                            nc.tensor.matmul(psq[m][:], xqT[j][:, m * 128:(m + 1) * 128], wt[:],
                                             start=(j == 0), stop=(j == NHT - 1))
                    for m in range(NT):
                        par = m % 2
                        if ng < 5:
                            qc_t = pQC.tile([128, 512], f32, tag="qc")
                            nc.vector.tensor_scalar(qc_t[:], psq[m][:], d1s[m][:],
                                                    None, ALU.mult)
                            xv = qc_t[:].rearrange("p (h t d) -> p h t d", t=2, d=32)
                            xr = xv[:, :, 0, :]
                            xi = xv[:, :, 1, :]
                            cv = cosr[par][:].rearrange("p (h d) -> p h d", d=32)
                            sv = sinr[par][:].rearrange("p (h d) -> p h d", d=32)
                            ov = sends[m][:, ng * 512:(ng + 1) * 512].rearrange(
                                "p (h t d) -> p h t d", t=2, d=32)
                            o_r = ov[:, :, 0, :]
                            o_i = ov[:, :, 1, :]
                            ta = pRT.tile([128, 256], f32, tag="ta")
                            tb = pRT.tile([128, 256], f32, tag="tb")
                            tav = ta[:].rearrange("p (h d) -> p h d", d=32)
                            tbv = tb[:].rearrange("p (h d) -> p h d", d=32)
                            tc_ = pRT.tile([128, 256], f32, tag="tc")
                            td = pRT.tile([128, 256], f32, tag="td")
                            tcv = tc_[:].rearrange("p (h d) -> p h d", d=32)
                            tdv = td[:].rearrange("p (h d) -> p h d", d=32)
                            nc.vector.tensor_tensor(tav, xr, cv, ALU.mult)
                            nc.vector.tensor_tensor(tbv, xi, sv, ALU.mult)
                            nc.vector.tensor_tensor(o_r, tav, tbv, ALU.subtract)
                            nc.vector.tensor_tensor(tcv, xr, sv, ALU.mult)
                            nc.vector.tensor_tensor(tdv, xi, cv, ALU.mult)
                            nc.vector.tensor_tensor(o_i, tcv, tdv, ALU.add)
                        else:
                            nc.vector.tensor_scalar(sends[m][:, ng * 512:(ng + 1) * 512],
                                                    psq[m][:], d1s[m][:], None, ALU.mult)

                    if ng in (1, 3):
                        # a head-pair's q chunks complete: scatter + fire its AllToAll
                        hp = ng // 2
                        for m in range(NT):
                            b = m // 2
                            par = m % 2
                            base = par * 128 * 256 + b * 128
                            for dh in range(2):
                                nc.sync.dma_start(
                                    out=_dap(aq_i[hp][:], base + dh * 4 * SC * 256,
                                             [[256, 128], [SC * 256, 4], [1, 128]]),
                                    in_=sends[m][:, hp * 1024 + dh * 512:
                                                 hp * 1024 + (dh + 1) * 512].rearrange(
                                        "p (j c) -> p j c", j=4))
                        nc.gpsimd.collective_compute(
                            "AllToAll", ALU.bypass, replica_groups=[list(range(C))],
                            ins=[aq_i[hp][:].opt()], outs=[aq_o[hp][:].opt()])
                        if ng == 1:
                            # ACT is idle until the first exp: issue the
                            # hp0/b0 q transpose-load right behind its a2a
                            load_qt(0, 0, nc.scalar)
                    elif ng == 4:
                        # k chunk complete: scatter + k AllToAll (dup built locally)
                        for m in range(NT):
                            b = m // 2
                            par = m % 2
                            base = par * 128 * 128 + b * 64
                            nc.sync.dma_start(
                                out=_dap(ak_i[:], base,
                                         [[128, 128], [SC * 128, 8], [1, 64]]),
                                in_=sends[m][:, 2048:2560].rearrange(
                                    "p (j c) -> p j c", j=8))
                        nc.gpsimd.collective_compute(
                            "AllToAll", ALU.bypass, replica_groups=[list(range(C))],
                            ins=[ak_i[:].opt()], outs=[ak_o[:].opt()])
                    elif ng == 5:
                        # v chunk complete: scatter + v AllToAll
                        for m in range(NT):
                            b = m // 2
                            par = m % 2
                            base = par * 128 * 128 + b * 64
                            nc.sync.dma_start(
                                out=_dap(av_i[:], base,
                                         [[128, 128], [SC * 128, 8], [1, 64]]),
                                in_=sends[m][:, 2560:3072].rearrange("p (j c) -> p j c", j=8))
                        nc.gpsimd.collective_compute(
                            "AllToAll", ALU.bypass, replica_groups=[list(range(C))],
                            ins=[av_i[:].opt()], outs=[av_o[:].opt()])

                # all attention-side loads issue from Sync after the stripe
                # issues; a2i scatters live on gpsimd so a blocking load here
                # stalls nothing
                load_kb(nc.sync)
                load_vas(nc.sync)
                load_qt(1, 0, nc.sync)
                load_qt(0, 1, nc.sync)
                load_qt(1, 1, nc.sync)

            # ================= Stage C: attention =========================
            # and Stage D: out projection, interleaved per-batch.
            with ExitStack() as sc:
                pEX = sc.enter_context(tc.tile_pool(name="pEX", bufs=28))
                pOB = sc.enter_context(tc.tile_pool(name="pOB", bufs=4))
                pR = sc.enter_context(tc.tile_pool(name="pR", bufs=3))
                pD = sc.enter_context(tc.tile_pool(name="pD", bufs=3))
                pXT2 = sc.enter_context(tc.tile_pool(name="pXT2", bufs=NHT + 2))
                pW2 = sc.enter_context(tc.tile_pool(name="pW2", bufs=32))
                pO = sc.enter_context(tc.tile_pool(name="pO", bufs=3))
                pDs = sc.enter_context(tc.tile_pool(name="pDs", bufs=2))

                def attn_unit(b, hp):
                    """Causal attention for batch b, q-head-pair hp.
                    AV matmuls of qc-1 are interleaved between QK tiles of qc
                    at ~2:1 so the exp stream always has a fresh score tile and
                    the PE never drains during an AV block."""
                    KH0, KH1 = KBs[b]
                    vab = vas[b]
                    qTx = qTs[b][hp]

                    def av_gen(qc, exs):
                        for h in range(2):
                            hg = hp * 2 + h
                            poT = psA.tile([128, 512], f32, tag="acc",
                                           name=f"poT_{b}_{hp}_{qc}_{h}")
                            last = 4 * qc + 3
                            for kt in range(last + 1):
                                dpos = max(0, kt * 128 - qc * 512)
                                nc.tensor.matmul(
                                    poT[0:65, dpos:512],
                                    vab[kt][:, 0:65],
                                    exs[kt][:, h * 512 + dpos:(h + 1) * 512],
                                    start=(kt == 0), stop=(kt == last),
                                    skip_group_check=(kt > 0))
                                yield
                            rs = pR.tile([1, 512], f32, tag="rs")
                            nc.vector.tensor_copy(rs[0:1, :], poT[64:65, 0:512])
                            rq = pR.tile([1, 512], f32, tag="rq")
                            nc.vector.reciprocal_approx_fast(out=rq[0:1, :],
                                                             in_=rs[0:1, :])
                            bc = pR.tile([64, 512], f32, tag="bc")
                            nc.gpsimd.partition_broadcast(bc[:, :], rq[0:1, :], channels=64)
                            nrm = pOB.tile([64, 512], bf16, tag="nrm",
                                           name=f"nrm_{b}_{hp}_{qc}_{h}")
                            nc.vector.tensor_tensor(nrm[:], poT[0:64, 0:512], bc[:], ALU.mult)
                            for par in range(2):
                                nc.gpsimd.dma_start(
                                    out=_dap(a2i[b][:],
                                             (((2 * qc + par) * 256 + hg * 64) * 256),
                                             [[256, 64], [1, 256]]),
                                    in_=nrm[:, par * 256:(par + 1) * 256])
                            yield

                    gen = None
                    for qc in (3, 2, 1, 0):
                        exs = []
                        for kt in range(4 * qc + 4):
                            dpos = max(0, kt * 128 - qc * 512)
                            st = psS.tile([128, 1024], f32, tag="st",
                                          name=f"st_{b}_{hp}_{qc}_{kt}")
                            nc.tensor.matmul(
                                st[:, dpos:512],
                                KH0[0:64, kt * 128:(kt + 1) * 128],
                                qTx[0:64, qc * 512 + dpos:(qc + 1) * 512],
                                start=True, stop=True)
                            nc.tensor.matmul(
                                st[:, 512 + dpos:1024],
                                KH1[64:128, kt * 128:(kt + 1) * 128],
                                qTx[64:128, qc * 512 + dpos:(qc + 1) * 512],
                                start=True, stop=True, tile_position=(64, 0))
                            ex = pEX.tile([128, 1024], bf16, tag="ex",
                                          name=f"ex_{b}_{hp}_{qc}_{kt}")
                            stv = st[:].rearrange("p (h q) -> p h q", h=2)[:, :, dpos:512]
                            exv = ex[:].rearrange("p (h q) -> p h q", h=2)[:, :, dpos:512]
                            nc.scalar.activation(out=exv, in_=stv, func=FT.Exp, scale=0.125)
                            if kt >= 4 * qc:
                                for h in range(2):
                                    sl = ex[:, h * 512 + dpos:h * 512 + dpos + 128]
                                    nc.vector.tensor_tensor(sl, sl, trim[:], ALU.mult)
                            exs.append(ex)
                            if gen is not None:
                                next(gen, None)
                                next(gen, None)
                        if gen is not None:
                            for _ in gen:
                                pass
                        gen = av_gen(qc, exs)
                    for _ in gen:
                        pass

                xq2T = [[None] * NHT for _ in range(B)]
                d2col = [[None] * 2 for _ in range(B)]

                x2tt = {}

                def load_x2t(b, eng):
                    x2ts = []
                    for j in range(NHT):
                        xt = pXT2.tile([128, SC], bf16, tag="x2t", name=f"x2t_{b}_{j}")
                        eng.dma_start(out=xt[:], in_=a2o[b][j * 128:(j + 1) * 128, :])
                        x2ts.append(xt)
                    x2tt[b] = x2ts

                def d_quant(b, reverse=False):
                    """Per-token absmax + int8 quantization of x^T for batch b.
                    Produces xq2T tiles in the same j order d_mm consumes."""
                    x2ts = x2tt[b]
                    jorder = list(range(NHT - 1, -1, -1)) if reverse else list(range(NHT))
                    # elementwise |.| max-accumulate on DVE, then one cross-partition
                    # absmax on gpsimd
                    acc = pDs.tile([128, SC], bf16, tag="aacc", name=f"aacc_{b}", bufs=2)
                    nc.vector.scalar_tensor_tensor(acc[:], x2ts[0][:], -1.0, x2ts[0][:],
                                                   ALU.mult, ALU.max)
                    for j in range(1, NHT):
                        nc.vector.scalar_tensor_tensor(acc[:], x2ts[j][:], -1.0, acc[:],
                                                       ALU.mult, ALU.max)
                        nc.vector.tensor_tensor(acc[:], acc[:], x2ts[j][:], ALU.max)
                    pm = pDs.tile([128, SC], f32, tag="pm", bufs=2)
                    nc.gpsimd.partition_all_reduce(pm[:], acc[:], 128,
                                                   bass_isa.ReduceOp.absmax)
                    mp = pDs.tile([1, SC], f32, tag="mprow", name=f"mprow_{b}", bufs=1)
                    nc.vector.tensor_scalar(mp[0:1, :], pm[0:1, :], 1e-5, None, ALU.max)
                    # d2 row -> DRAM -> read back as a column (per 128-token tile)
                    d2r = pDs.tile([1, SC], f32, tag="d2row", name=f"d2row_{b}", bufs=1)
                    nc.vector.tensor_scalar(d2r[0:1, :], mp[0:1, :],
                                            sw2b[0:1, 0:1], None, ALU.mult)
                    nc.sync.dma_start(out=d2dr[b:b + 1, :], in_=d2r[0:1, :])
                    for m2 in range(2):
                        dc = pDs.tile([128, 1], f32, tag="d2c", name=f"d2c_{b}_{m2}", bufs=4)
                        nc.sync.dma_start(
                            out=dc[:],
                            in_=_dap(d2dr[:], b * SC + m2 * 128, [[1, 128], [SC * B, 1]]))
                        d2col[b][m2] = dc
                    # scale row 127/max -> broadcast to all partitions
                    scr = pDs.tile([1, SC], f32, tag="scrow", name=f"scrow_{b}", bufs=1)
                    nc.vector.reciprocal_approx_fast(out=scr[0:1, :], in_=mp[0:1, :])
                    sc2 = pDs.tile([1, SC], f32, tag="scrow2", name=f"scrow2_{b}", bufs=1)
                    nc.vector.tensor_scalar(sc2[0:1, :], scr[0:1, :], 127.0, None, ALU.mult)
                    scb = pDs.tile([128, SC], f32, tag="scb", name=f"scb_{b}")
                    nc.gpsimd.partition_broadcast(scb[:, :], sc2[0:1, :], channels=128)
                    tqs = {}
                    for j in jorder:
                        tq = pD.tile([128, SC], f32, tag="tq", bufs=3)
                        nc.vector.tensor_tensor(tq[:], x2ts[j][:], scb[:], ALU.mult)
                        tq2 = pD.tile([128, SC], f32, tag="tq2", bufs=3)
                        nc.scalar.add(tq2[:], tq[:], magicb[:])
                        tqs[j] = tq2
                    for j in jorder:
                        xqt = pXT2.tile([128, SC], bf16, tag="xq2t", name=f"xq2t_{b}_{j}")
                        nc.vector.tensor_scalar(xqt[:], tqs[j][:], MAGIC, None, ALU.subtract)
                        xq2T[b][j] = xqt

                wt_cache = {}

                def d_mm(b, reverse):
                    """Out projection matmuls + dequant + store for batch b.
                    Pass 2 runs in reverse stripe order and reuses the last
                    pW2-ring stripes of pass 1 that are still resident."""
                    ngs = range(3, -1, -1) if reverse else range(4)
                    js = range(NHT - 1, -1, -1) if reverse else range(NHT)
                    for ng in ngs:
                        ps2 = [psA.tile([128, 512], f32, tag="acc", name=f"ps2_{b}_{ng}_{m2}")
                               for m2 in range(2)]
                        first = True
                        for j in js:
                            if (ng, j) in wt_cache:
                                wt = wt_cache.pop((ng, j))
                            else:
                                wt = pW2.tile([128, 512], f8, tag="w2")
                                r0 = (ng * NHT + j) * 128
                                nc.sync.dma_start(out=wt[:], in_=WQ2[r0:r0 + 128, :])
                                if not reverse and (ng == 3 or (ng == 2 and j >= 10)):
                                    wt_cache[(ng, j)] = wt
                            for m2 in range(2):
                                nc.tensor.matmul(
                                    ps2[m2][:],
                                    xq2T[b][j][:, m2 * 128:(m2 + 1) * 128],
                                    wt[:], start=first, stop=(j == (0 if reverse else NHT - 1)))
                            first = False
                        for m2 in range(2):
                            ot = pO.tile([128, 512], f32, tag="ot")
                            nc.scalar.mul(ot[:], ps2[m2][:], d2col[b][m2][:])
                            r0 = (b * 2 + m2) * 128
                            for ch in range(2):
                                nc.sync.dma_start(
                                    out=OUT[r0:r0 + 128,
                                            ng * 512 + ch * 256:ng * 512 + (ch + 1) * 256],
                                    in_=ot[:, ch * 256:(ch + 1) * 256])

                def fire_a2a2(b):
                    nc.gpsimd.collective_compute(
                        "AllToAll", ALU.bypass, replica_groups=[list(range(C))],
                        ins=[a2i[b][:].opt()], outs=[a2o[b][:].opt()])

                attn_unit(0, 0)
                attn_unit(0, 1)
                fire_a2a2(0)
                load_x2t(0, nc.sync)
                attn_unit(1, 0)
                d_quant(0)
                attn_unit(1, 1)
                fire_a2a2(1)
                load_x2t(1, nc.gpsimd)
                d_mm(0, False)
                d_quant(1, reverse=True)
                d_mm(1, True)

    nc.compile()
    return nc


_NC_CACHE = {}


def _get_nc():
    if "nc" not in _NC_CACHE:
        _NC_CACHE["nc"] = build_nc()
    return _NC_CACHE["nc"]


def _stripe(wt, nchunk):
    """[H, O] -> [(O//512)*16*128, 512] contiguous (ng, j)-stripe layout."""
    Hh, O = wt.shape
    a = wt.reshape(NHT, 128, O // 512, 512)          # [j, h, ng, c]
    a = a.transpose(2, 0, 1, 3)                      # [ng, j, h, c]
    return np.ascontiguousarray(a.reshape(-1, 512))


def kernel(x, w_norm, w_qkv, w_out):
    x = np.asarray(x, dtype=np.float32)
    w_norm = np.asarray(w_norm, dtype=np.float32)
    w_qkv = np.asarray(w_qkv, dtype=np.float32)
    w_out = np.asarray(w_out, dtype=np.float32)

    def tern(w):
        ws = np.float32(1.0) / np.clip(np.mean(np.abs(w)), np.float32(1e-5), None).astype(np.float32)
        wq = np.clip(np.round(w * ws), -1.0, 1.0).astype(np.float32)
        return wq, (np.float32(1.0) / ws).astype(np.float32)

    wq1, s_w1 = tern(w_qkv)
    wq2, s_w2 = tern(w_out)
    # permute q head blocks: new col hp*1024 + dest*128 + (h%2)*64 + d
    hperm = np.empty(NH, np.int64)
    for h in range(NH):
        hperm[(h % 4) // 2 * 16 + (h // 4) * 2 + (h % 2)] = h
    qperm = (hperm[:, None] * HD + np.arange(HD)[None, :]).reshape(-1)
    wq1p = wq1.copy()
    wq1p[:NH * HD] = wq1[qperm]
    wq1t = _stripe(np.ascontiguousarray(wq1p.T), QKV_O // 512).astype(ml_dtypes.float8_e4m3fn)
    wq2t = _stripe(np.ascontiguousarray(wq2.T), H // 512).astype(ml_dtypes.float8_e4m3fn)

    inv_freq = (1.0 / THETA ** (np.arange(0, HD, 2, dtype=np.float32) / HD)).astype(np.float32)
    t_pos = np.arange(S, dtype=np.float32)
    freqs = t_pos[:, None] * inv_freq[None, :]
    cos_full = np.cos(freqs).astype(np.float32)
    sin_full = np.sin(freqs).astype(np.float32)

    trimask = np.triu(np.ones((128, 128), np.float32)).astype(ml_dtypes.bfloat16)
    sw1 = np.array([[s_w1 / np.float32(127.0)]], dtype=np.float32)
    sw2 = np.array([[s_w2 / np.float32(127.0)]], dtype=np.float32)
    wn2d = w_norm.reshape(1, H)

    in_maps = []
    for i in range(C):
        xc = np.ascontiguousarray(
            np.concatenate([x[0, i * SC:(i + 1) * SC, :], x[1, i * SC:(i + 1) * SC, :]], axis=0))
        in_maps.append({
            "x": xc,
            "wn": wn2d,
            "wq1t": wq1t,
            "wq2t": wq2t,
            "cosb": np.ascontiguousarray(np.tile(cos_full[i * SC:(i + 1) * SC, :], (1, 8))),
            "sinb": np.ascontiguousarray(np.tile(sin_full[i * SC:(i + 1) * SC, :], (1, 8))),
            "trimask": trimask,
            "sw1": sw1,
            "sw2": sw2,
        })

    nc = _get_nc()
    res = bass_utils.run_bass_kernel_spmd(nc, in_maps, core_ids=list(range(C)))

    out = np.empty((B, S, H), dtype=np.float32)
    for i in range(C):
        ci = res.results[i]["out"]
        for b in range(B):
            out[b, i * SC:(i + 1) * SC, :] = ci[b * SC:(b + 1) * SC, :]
    return out


# revision 56
# speedup vs baseline: 1.0116x; 1.0116x over previous
"""Distributed Trainium2 Bass kernel for BitNet-style attention block.

Sharding: sequence-parallel projections + (batch x kv-head) parallel attention,
stitched with per-batch AllToAll collectives (split so comm overlaps compute).

Per core (core i):
  A. RMSNorm + per-token absmax quantization of its 512-token chunk.
  B. qkv projection as exact integer bf16 matmul against host-prequantized
     ternary weights, dequant, RoPE on q/k, scatter into per-batch A2A bufs.
  C. AllToAll #1 (k/v/qA/qB fired as their data completes) -> core i holds
     full-sequence q/k/v for kv-head i of each batch; causal attention with
     transposed scores, exp on ACT, and a TRANSPOSED attention*V (stationary
     [v|ones], moving probabilities, 512-wide) that emits out^T [d, qi] plus
     rowsums; normalization via gpsimd partition-broadcast of 1/rowsum.
  D. AllToAll #2 ships out^T (hidden-major rows) -> core i holds x^T
     [2048 hidden, 256 tok] per batch; per-token quantization via gpsimd
     partition all-reduce absmax, integer matmul with ternary output weights
     (no PE transposes needed - x^T rows are already the contract dim).
"""
import sys
sys.path.insert(0, "/opt/trn_rl_repo")
import numpy as np
import ml_dtypes
import concourse.bass as bass
import concourse.tile as tile
from concourse import bacc, mybir
from concourse import bass_utils
from concourse import bass_isa
from concourse.masks import make_identity

f32 = mybir.dt.float32
bf16 = mybir.dt.bfloat16
f8 = mybir.dt.float8e4
FT = mybir.ActivationFunctionType
ALU = mybir.AluOpType

B, S, H = 2, 2048, 2048
NH, NKV, HD = 32, 8, 64
G = NH // NKV                    # 4
QKV_O = (NH + 2 * NKV) * HD      # 3072
EPS = 1e-5
THETA = 10000.0
C = 8
SC = S // C                      # 256 positions per core
TOK = B * SC                     # 512 token rows per core
MAGIC = float(1.5 * 2.0 ** 23)   # RNE integer rounding for |v| < 2^22
NT = TOK // 128                  # 4 token tiles
NHT = H // 128                   # 16 h-tiles
NKT = S // 128                   # 16 kj tiles

# a2a1 split into four column-group collectives fired as their data completes:
#   k: per-batch [k 64] -> [8, 128, 128]  (dup built locally via 2 transpose loads)
#   v: per-batch [v 64] -> [8, 128, 128]
#   qA/qB: per-batch head-pair [2 heads = 128] -> [8, 256, 256] each
# q heads are permuted host-side: col hp*1024 + dest*128 + hh*64 + d
# a2a2 is transposed: rows = dest*256 + head*64 + d, cols = dest's 256 tokens.


def _dap(t_ap, extra, dims):
    return bass.AP(tensor=t_ap.tensor, offset=t_ap.offset + extra, ap=[list(d) for d in dims])


def build_nc():
    nc = bacc.Bacc("TRN2", target_bir_lowering=False, debug=False, num_devices=C)

    x_in = nc.dram_tensor("x", [TOK, H], f32, kind="ExternalInput")
    wn_in = nc.dram_tensor("wn", [1, H], f32, kind="ExternalInput")
    # contiguous stripes: row ((ng*16+j)*128 + h_local), 512 cols each
    wq1t_in = nc.dram_tensor("wq1t", [(QKV_O // 512) * NHT * 128, 512], bf16, kind="ExternalInput")
    wq2t_in = nc.dram_tensor("wq2t", [(H // 512) * NHT * 128, 512], bf16, kind="ExternalInput")
    cos_in = nc.dram_tensor("cosb", [SC, 8 * 32], f32, kind="ExternalInput")
    sin_in = nc.dram_tensor("sinb", [SC, 8 * 32], f32, kind="ExternalInput")
    tri_in = nc.dram_tensor("trimask", [128, 128], bf16, kind="ExternalInput")
    sw1_in = nc.dram_tensor("sw1", [1, 1], f32, kind="ExternalInput")
    sw2_in = nc.dram_tensor("sw2", [1, 1], f32, kind="ExternalInput")
    out_ext = nc.dram_tensor("out", [TOK, H], f32, kind="ExternalOutput")

    X = x_in.ap()
    WQ1 = wq1t_in.ap()
    WQ2 = wq2t_in.ap()
    OUT = out_ext.ap()

    with tile.TileContext(nc) as tc:
        from contextlib import ExitStack
        with ExitStack() as top:
            dram = top.enter_context(tc.tile_pool(name="dram", bufs=1, space="DRAM"))
            const = top.enter_context(tc.tile_pool(name="const", bufs=1))
            smalls = top.enter_context(tc.tile_pool(name="smalls", bufs=1))
            psA = top.enter_context(tc.tile_pool(name="psA", bufs=4, space="PSUM"))
            psS = top.enter_context(tc.tile_pool(name="psS", bufs=2, space="PSUM"))

            # ---------------- DRAM scratch ----------------
            aq_i = [dram.tile([C * SC, 256], bf16, name=f"aq_i{hp}") for hp in range(2)]
            aq_o = [dram.tile([C * SC, 256], bf16, name=f"aq_o{hp}") for hp in range(2)]
            ak_i = dram.tile([C * SC, 128], bf16, name="ak_i")
            ak_o = dram.tile([C * SC, 128], bf16, name="ak_o")
            av_i = dram.tile([C * SC, 128], bf16, name="av_i")
            av_o = dram.tile([C * SC, 128], bf16, name="av_o")
            a2i = [dram.tile([C * SC, G * HD], bf16, name=f"a2i_{b}") for b in range(B)]
            a2o = [dram.tile([C * SC, G * HD], bf16, name=f"a2o_{b}") for b in range(B)]
            d2dr = dram.tile([B, SC], f32, name="d2dr")

            # ---------------- constants ----------------
            wnorm_b = const.tile([128, H], f32)
            wnr = const.tile([1, H], f32)
            nc.sync.dma_start(out=wnr[0:1, :], in_=wn_in.ap()[0:1, :])
            nc.gpsimd.partition_broadcast(wnorm_b[:, :], wnr[0:1, :], channels=128)
            trim = const.tile([128, 128], bf16)
            nc.sync.dma_start(out=trim[:], in_=tri_in.ap()[:, :])
            sw1b = const.tile([128, 1], f32)
            nc.sync.dma_start(out=sw1b[:], in_=_dap(sw1_in.ap(), 0, [[0, 128], [1, 1]]))
            sw2b = const.tile([128, 1], f32)
            nc.sync.dma_start(out=sw2b[:], in_=_dap(sw2_in.ap(), 0, [[0, 128], [1, 1]]))
            epsb = const.tile([128, 1], f32)
            nc.vector.memset(epsb[:], EPS)
            magicb = const.tile([128, 1], f32)
            nc.vector.memset(magicb[:], MAGIC)
            ident = const.tile([128, 128], bf16)
            make_identity(nc, ident[:])

            d1s = [smalls.tile([128, 1], f32, name=f"d1_{m}") for m in range(NT)]

            xqT_pool = top.enter_context(tc.tile_pool(name="xqT", bufs=NHT))
            pQT = top.enter_context(tc.tile_pool(name="pQT", bufs=4))
            pKT = top.enter_context(tc.tile_pool(name="pKT", bufs=3))
            pVA = top.enter_context(tc.tile_pool(name="pVA", bufs=2 * NKT))

            # ================= Stage A: RMSNorm + quantize =================
            with ExitStack() as sa:
                pA = sa.enter_context(tc.tile_pool(name="pA", bufs=2))
                pXQ = sa.enter_context(tc.tile_pool(name="pXQ", bufs=NT))
                pSc = sa.enter_context(tc.tile_pool(name="pASc", bufs=4))
                xqms = []
                for m in range(NT):
                    xa = pA.tile([128, H], f32, tag="xa")
                    for xc in range(8):
                        nc.sync.dma_start(out=xa[:, xc * 256:(xc + 1) * 256],
                                          in_=X[m * 128:(m + 1) * 128, xc * 256:(xc + 1) * 256])
                    sq = pA.tile([128, H], f32, tag="sq")
                    ssq = pSc.tile([128, 1], f32, tag="ssq")
                    nc.scalar.activation(out=sq[:], in_=xa[:], func=FT.Square, accum_out=ssq[:])
                    xw = pA.tile([128, H], f32, tag="xw")
                    nc.vector.tensor_tensor(xw[:], xa[:], wnorm_b[:], ALU.mult)
                    std = pSc.tile([128, 1], f32, tag="std")
                    nc.scalar.activation(out=std[:], in_=ssq[:], func=FT.Sqrt,
                                         bias=epsb[:], scale=1.0 / H)
                    rstd = pSc.tile([128, 1], f32, tag="rstd")
                    nc.vector.reciprocal(rstd[:], std[:])
                    mx = pSc.tile([128, 1], f32, tag="mx")
                    nc.vector.tensor_reduce(mx[:], xw[:], mybir.AxisListType.X, ALU.max,
                                            apply_absolute_value=True)
                    mp = pSc.tile([128, 1], f32, tag="mp")
                    nc.vector.tensor_scalar(mp[:], mx[:], rstd[:], 1e-5, ALU.mult, ALU.max)
                    nc.vector.tensor_tensor(d1s[m][:], mp[:], sw1b[:], ALU.mult)
                    rmp = pSc.tile([128, 1], f32, tag="rmp")
                    nc.vector.reciprocal(rmp[:], mp[:])
                    csc = pSc.tile([128, 1], f32, tag="csc")
                    nc.vector.tensor_scalar(csc[:], rmp[:], rstd[:], 127.0, ALU.mult, ALU.mult)
                    t1 = pA.tile([128, H], f32, tag="t1")
                    nc.scalar.activation(out=t1[:], in_=xw[:], func=FT.Identity,
                                         scale=csc[:], bias=magicb[:])
                    xqm = pXQ.tile([128, H], bf16, tag="xqm", name=f"xqm_{m}")
                    nc.vector.tensor_scalar(xqm[:], t1[:], MAGIC, None, ALU.subtract)
                    xqms.append(xqm)

                # transposed activations via PE (keeps the DMA queues free)
                xqT = []
                for j in range(NHT):
                    t = xqT_pool.tile([128, TOK], bf16, name=f"xqT_{j}", tag="xqT")
                    xqT.append(t)
                for m in range(NT):
                    for j in range(NHT):
                        tp = psS.tile([128, 128], bf16, tag="st", name=f"tp_{m}_{j}")
                        nc.tensor.transpose(tp[:], xqms[m][:, j * 128:(j + 1) * 128], ident[:])
                        nc.vector.tensor_copy(xqT[j][:, m * 128:(m + 1) * 128], tp[:])

            # ================= Stage B: qkv matmul + RoPE + scatter ========
            with ExitStack() as sb:
                pW = sb.enter_context(tc.tile_pool(name="pW", bufs=28))
                pQC = sb.enter_context(tc.tile_pool(name="pQC", bufs=7))
                pRT = sb.enter_context(tc.tile_pool(name="pRT", bufs=3))
                pSend = sb.enter_context(tc.tile_pool(name="pSend", bufs=NT))
                pCos = sb.enter_context(tc.tile_pool(name="pCos", bufs=1))

                cosr = []
                sinr = []
                for par in range(2):
                    ct = pCos.tile([128, 8 * 32], f32, name=f"cosr_{par}")
                    nc.sync.dma_start(out=ct[:], in_=cos_in.ap()[par * 128:(par + 1) * 128, :])
                    st_ = pCos.tile([128, 8 * 32], f32, name=f"sinr_{par}")
                    nc.sync.dma_start(out=st_[:], in_=sin_in.ap()[par * 128:(par + 1) * 128, :])
                    cosr.append(ct)
                    sinr.append(st_)

                sends = [pSend.tile([128, QKV_O], bf16, name=f"sends_{m}", tag="sends")
                         for m in range(NT)]

                qTs = [[None] * 2 for _ in range(B)]
                KBs = [None] * B
                vas = [[None] * NKT for _ in range(B)]

                def load_kb(eng):
                    # KBoth rows 0:64 = k(b0), rows 64:128 = k(b1); per-batch
                    # dup tiles give each batch both PE row halves.
                    KBoth = pKT.tile([128, S], bf16, name="KBoth", tag="kT")
                    KB0d = pKT.tile([128, S], bf16, name="KB0d", tag="kT")
                    KB1d = pKT.tile([128, S], bf16, name="KB1d", tag="kT")
                    for c4 in range(4):
                        cs = slice(c4 * 512, (c4 + 1) * 512)
                        eng.dma_start(out=KBoth[:, cs],
                                      in_=ak_o[c4 * 512:(c4 + 1) * 512, :],
                                      transpose=True)
                        nc.gpsimd.dma_start(out=KB0d[64:128, cs], in_=KBoth[0:64, cs])
                        nc.gpsimd.dma_start(out=KB1d[0:64, cs], in_=KBoth[64:128, cs])
                    KBs[0] = (KBoth, KB0d)
                    KBs[1] = (KB1d, KBoth)

                def load_qt(hp, b, eng):
                    t = pQT.tile([128, S], bf16, name=f"qT_{b}_{hp}", tag="qT")
                    for c4 in range(4):
                        eng.dma_start(
                            out=t[:, c4 * 512:(c4 + 1) * 512],
                            in_=aq_o[hp][c4 * 512:(c4 + 1) * 512,
                                         b * 128:(b + 1) * 128],
                            transpose=True)
                    qTs[b][hp] = t

                def load_vas(eng):
                    for b in range(B):
                        for kt in range(NKT):
                            t = pVA.tile([128, 65], bf16, name=f"va_{b}_{kt}", tag="va")
                            eng.dma_start(
                                out=t[:, 0:64],
                                in_=av_o[kt * 128:(kt + 1) * 128, b * 64:(b + 1) * 64])
                            nc.vector.memset(t[:, 64:65], 1.0)
                            vas[b][kt] = t

                for ng in (4, 0, 1, 5, 2, 3):
                    psq = [psA.tile([128, 512], f32, tag="acc", name=f"qkvp_{ng}_{m}")
                           for m in range(NT)]
                    for j in range(NHT):
                        wt = pW.tile([128, 512], bf16, tag="w1")
                        r0 = (ng * NHT + j) * 128
                        nc.sync.dma_start(out=wt[:], in_=WQ1[r0:r0 + 128, :])
                        for m in range(NT):
                            nc.tensor.matmul(psq[m][:], xqT[j][:, m * 128:(m + 1) * 128], wt[:],
                                             start=(j == 0), stop=(j == NHT - 1))
                    for m in range(NT):
                        par = m % 2
                        if ng < 5:
                            qc_t = pQC.tile([128, 512], f32, tag="qc")
                            nc.vector.tensor_scalar(qc_t[:], psq[m][:], d1s[m][:],
                                                    None, ALU.mult)
                            xv = qc_t[:].rearrange("p (h t d) -> p h t d", t=2, d=32)
                            xr = xv[:, :, 0, :]
                            xi = xv[:, :, 1, :]
                            cv = cosr[par][:].rearrange("p (h d) -> p h d", d=32)
                            sv = sinr[par][:].rearrange("p (h d) -> p h d", d=32)
                            ov = sends[m][:, ng * 512:(ng + 1) * 512].rearrange(
                                "p (h t d) -> p h t d", t=2, d=32)
                            o_r = ov[:, :, 0, :]
                            o_i = ov[:, :, 1, :]
                            ta = pRT.tile([128, 256], f32, tag="ta")
                            tb = pRT.tile([128, 256], f32, tag="tb")
                            tav = ta[:].rearrange("p (h d) -> p h d", d=32)
                            tbv = tb[:].rearrange("p (h d) -> p h d", d=32)
                            tc_ = pRT.tile([128, 256], f32, tag="tc")
                            td = pRT.tile([128, 256], f32, tag="td")
                            tcv = tc_[:].rearrange("p (h d) -> p h d", d=32)
                            tdv = td[:].rearrange("p (h d) -> p h d", d=32)
                            nc.vector.tensor_tensor(tav, xr, cv, ALU.mult)
                            nc.vector.tensor_tensor(tbv, xi, sv, ALU.mult)
                            nc.vector.tensor_tensor(o_r, tav, tbv, ALU.subtract)
                            nc.vector.tensor_tensor(tcv, xr, sv, ALU.mult)
                            nc.vector.tensor_tensor(tdv, xi, cv, ALU.mult)
                            nc.vector.tensor_tensor(o_i, tcv, tdv, ALU.add)
                        else:
                            nc.vector.tensor_scalar(sends[m][:, ng * 512:(ng + 1) * 512],
                                                    psq[m][:], d1s[m][:], None, ALU.mult)

                    if ng in (1, 3):
                        # a head-pair's q chunks complete: scatter + fire its AllToAll
                        hp = ng // 2
                        for m in range(NT):
                            b = m // 2
                            par = m % 2
                            base = par * 128 * 256 + b * 128
                            for dh in range(2):
                                nc.sync.dma_start(
                                    out=_dap(aq_i[hp][:], base + dh * 4 * SC * 256,
                                             [[256, 128], [SC * 256, 4], [1, 128]]),
                                    in_=sends[m][:, hp * 1024 + dh * 512:
                                                 hp * 1024 + (dh + 1) * 512].rearrange(
                                        "p (j c) -> p j c", j=4))
                        nc.gpsimd.collective_compute(
                            "AllToAll", ALU.bypass, replica_groups=[list(range(C))],
                            ins=[aq_i[hp][:].opt()], outs=[aq_o[hp][:].opt()])
                        if ng == 1:
                            # ACT is idle until the first exp: issue the
                            # hp0/b0 q transpose-load right behind its a2a
                            load_qt(0, 0, nc.scalar)
                    elif ng == 4:
                        # k chunk complete: scatter + k AllToAll (dup built locally)
                        for m in range(NT):
                            b = m // 2
                            par = m % 2
                            base = par * 128 * 128 + b * 64
                            nc.sync.dma_start(
                                out=_dap(ak_i[:], base,
                                         [[128, 128], [SC * 128, 8], [1, 64]]),
                                in_=sends[m][:, 2048:2560].rearrange(
                                    "p (j c) -> p j c", j=8))
                        nc.gpsimd.collective_compute(
                            "AllToAll", ALU.bypass, replica_groups=[list(range(C))],
                            ins=[ak_i[:].opt()], outs=[ak_o[:].opt()])
                    elif ng == 5:
                        # v chunk complete: scatter + v AllToAll
                        for m in range(NT):
                            b = m // 2
                            par = m % 2
                            base = par * 128 * 128 + b * 64
                            nc.sync.dma_start(
                                out=_dap(av_i[:], base,
                                         [[128, 128], [SC * 128, 8], [1, 64]]),
                                in_=sends[m][:, 2560:3072].rearrange("p (j c) -> p j c", j=8))
                        nc.gpsimd.collective_compute(
                            "AllToAll", ALU.bypass, replica_groups=[list(range(C))],
                            ins=[av_i[:].opt()], outs=[av_o[:].opt()])

                # all attention-side loads issue from Sync after the stripe
                # issues; a2i scatters live on gpsimd so a blocking load here
                # stalls nothing
                load_kb(nc.sync)
                load_vas(nc.sync)
                load_qt(1, 0, nc.sync)
                load_qt(0, 1, nc.sync)
                load_qt(1, 1, nc.sync)

            # ================= Stage C: attention =========================
            # and Stage D: out projection, interleaved per-batch.
            with ExitStack() as sc:
                pEX = sc.enter_context(tc.tile_pool(name="pEX", bufs=28))
                pOB = sc.enter_context(tc.tile_pool(name="pOB", bufs=4))
                pR = sc.enter_context(tc.tile_pool(name="pR", bufs=3))
                pD = sc.enter_context(tc.tile_pool(name="pD", bufs=3))
                pXT2 = sc.enter_context(tc.tile_pool(name="pXT2", bufs=NHT + 2))
                pW2 = sc.enter_context(tc.tile_pool(name="pW2", bufs=24))
                pO = sc.enter_context(tc.tile_pool(name="pO", bufs=3))
                pDs = sc.enter_context(tc.tile_pool(name="pDs", bufs=2))

                def attn_unit(b, hp):
                    """Causal attention for batch b, q-head-pair hp.
                    AV matmuls of qc-1 are interleaved between QK tiles of qc
                    at ~2:1 so the exp stream always has a fresh score tile and
                    the PE never drains during an AV block."""
                    KH0, KH1 = KBs[b]
                    vab = vas[b]
                    qTx = qTs[b][hp]

                    def av_gen(qc, exs):
                        for h in range(2):
                            hg = hp * 2 + h
                            poT = psA.tile([128, 512], f32, tag="acc",
                                           name=f"poT_{b}_{hp}_{qc}_{h}")
                            last = 4 * qc + 3
                            for kt in range(last + 1):
                                dpos = max(0, kt * 128 - qc * 512)
                                nc.tensor.matmul(
                                    poT[0:65, dpos:512],
                                    vab[kt][:, 0:65],
                                    exs[kt][:, h * 512 + dpos:(h + 1) * 512],
                                    start=(kt == 0), stop=(kt == last),
                                    skip_group_check=(kt > 0))
                                yield
                            rs = pR.tile([1, 512], f32, tag="rs")
                            nc.vector.tensor_copy(rs[0:1, :], poT[64:65, 0:512])
                            rq = pR.tile([1, 512], f32, tag="rq")
                            nc.vector.reciprocal_approx_fast(out=rq[0:1, :],
                                                             in_=rs[0:1, :])
                            bc = pR.tile([64, 512], f32, tag="bc")
                            nc.gpsimd.partition_broadcast(bc[:, :], rq[0:1, :], channels=64)
                            nrm = pOB.tile([64, 512], bf16, tag="nrm",
                                           name=f"nrm_{b}_{hp}_{qc}_{h}")
                            nc.vector.tensor_tensor(nrm[:], poT[0:64, 0:512], bc[:], ALU.mult)
                            for par in range(2):
                                nc.gpsimd.dma_start(
                                    out=_dap(a2i[b][:],
                                             (((2 * qc + par) * 256 + hg * 64) * 256),
                                             [[256, 64], [1, 256]]),
                                    in_=nrm[:, par * 256:(par + 1) * 256])
                            yield

                    gen = None
                    for qc in (3, 2, 1, 0):
                        exs = []
                        for kt in range(4 * qc + 4):
                            dpos = max(0, kt * 128 - qc * 512)
                            st = psS.tile([128, 1024], f32, tag="st",
                                          name=f"st_{b}_{hp}_{qc}_{kt}")
                            nc.tensor.matmul(
                                st[:, dpos:512],
                                KH0[0:64, kt * 128:(kt + 1) * 128],
                                qTx[0:64, qc * 512 + dpos:(qc + 1) * 512],
                                start=True, stop=True)
                            nc.tensor.matmul(
                                st[:, 512 + dpos:1024],
                                KH1[64:128, kt * 128:(kt + 1) * 128],
                                qTx[64:128, qc * 512 + dpos:(qc + 1) * 512],
                                start=True, stop=True, tile_position=(64, 0))
                            ex = pEX.tile([128, 1024], bf16, tag="ex",
                                          name=f"ex_{b}_{hp}_{qc}_{kt}")
                            stv = st[:].rearrange("p (h q) -> p h q", h=2)[:, :, dpos:512]
                            exv = ex[:].rearrange("p (h q) -> p h q", h=2)[:, :, dpos:512]
                            nc.scalar.activation(out=exv, in_=stv, func=FT.Exp, scale=0.125)
                            if kt >= 4 * qc:
                                for h in range(2):
                                    sl = ex[:, h * 512 + dpos:h * 512 + dpos + 128]
                                    nc.vector.tensor_tensor(sl, sl, trim[:], ALU.mult)
                            exs.append(ex)
                            if gen is not None:
                                next(gen, None)
                                next(gen, None)
                        if gen is not None:
                            for _ in gen:
                                pass
                        gen = av_gen(qc, exs)
                    for _ in gen:
                        pass

                xq2T = [[None] * NHT for _ in range(B)]
                d2col = [[None] * 2 for _ in range(B)]

                x2tt = {}

                def load_x2t(b, eng):
                    x2ts = []
                    for j in range(NHT):
                        xt = pXT2.tile([128, SC], bf16, tag="x2t", name=f"x2t_{b}_{j}")
                        eng.dma_start(out=xt[:], in_=a2o[b][j * 128:(j + 1) * 128, :])
                        x2ts.append(xt)
                    x2tt[b] = x2ts

                def d_quant(b, reverse=False):
                    """Per-token absmax + int8 quantization of x^T for batch b.
                    Produces xq2T tiles in the same j order d_mm consumes."""
                    x2ts = x2tt[b]
                    jorder = list(range(NHT - 1, -1, -1)) if reverse else list(range(NHT))
                    # elementwise |.| max-accumulate on DVE, then one cross-partition
                    # absmax on gpsimd
                    acc = pDs.tile([128, SC], bf16, tag="aacc", name=f"aacc_{b}", bufs=2)
                    nc.vector.scalar_tensor_tensor(acc[:], x2ts[0][:], -1.0, x2ts[0][:],
                                                   ALU.mult, ALU.max)
                    for j in range(1, NHT):
                        nc.vector.scalar_tensor_tensor(acc[:], x2ts[j][:], -1.0, acc[:],
                                                       ALU.mult, ALU.max)
                        nc.vector.tensor_tensor(acc[:], acc[:], x2ts[j][:], ALU.max)
                    pm = pDs.tile([128, SC], f32, tag="pm", bufs=2)
                    nc.gpsimd.partition_all_reduce(pm[:], acc[:], 128,
                                                   bass_isa.ReduceOp.absmax)
                    mp = pDs.tile([1, SC], f32, tag="mprow", name=f"mprow_{b}", bufs=1)
                    nc.vector.tensor_scalar(mp[0:1, :], pm[0:1, :], 1e-5, None, ALU.max)
                    # d2 row -> DRAM -> read back as a column (per 128-token tile)
                    d2r = pDs.tile([1, SC], f32, tag="d2row", name=f"d2row_{b}", bufs=1)
                    nc.vector.tensor_scalar(d2r[0:1, :], mp[0:1, :],
                                            sw2b[0:1, 0:1], None, ALU.mult)
                    nc.sync.dma_start(out=d2dr[b:b + 1, :], in_=d2r[0:1, :])
                    for m2 in range(2):
                        dc = pDs.tile([128, 1], f32, tag="d2c", name=f"d2c_{b}_{m2}", bufs=4)
                        nc.sync.dma_start(
                            out=dc[:],
                            in_=_dap(d2dr[:], b * SC + m2 * 128, [[1, 128], [SC * B, 1]]))
                        d2col[b][m2] = dc
                    # scale row 127/max -> broadcast to all partitions
                    scr = pDs.tile([1, SC], f32, tag="scrow", name=f"scrow_{b}", bufs=1)
                    nc.vector.reciprocal_approx_fast(out=scr[0:1, :], in_=mp[0:1, :])
                    sc2 = pDs.tile([1, SC], f32, tag="scrow2", name=f"scrow2_{b}", bufs=1)
                    nc.vector.tensor_scalar(sc2[0:1, :], scr[0:1, :], 127.0, None, ALU.mult)
                    scb = pDs.tile([128, SC], f32, tag="scb", name=f"scb_{b}")
                    nc.gpsimd.partition_broadcast(scb[:, :], sc2[0:1, :], channels=128)
                    tqs = {}
                    for j in jorder:
                        tq = pD.tile([128, SC], f32, tag="tq", bufs=3)
                        nc.vector.tensor_tensor(tq[:], x2ts[j][:], scb[:], ALU.mult)
                        tq2 = pD.tile([128, SC], f32, tag="tq2", bufs=3)
                        nc.scalar.add(tq2[:], tq[:], magicb[:])
                        tqs[j] = tq2
                    for j in jorder:
                        xqt = pXT2.tile([128, SC], bf16, tag="xq2t", name=f"xq2t_{b}_{j}")
                        nc.vector.tensor_scalar(xqt[:], tqs[j][:], MAGIC, None, ALU.subtract)
                        xq2T[b][j] = xqt

                wt_cache = {}

                def d_mm(b, reverse):
                    """Out projection matmuls + dequant + store for batch b.
                    Pass 2 runs in reverse stripe order and reuses the last
                    pW2-ring stripes of pass 1 that are still resident."""
                    ngs = range(3, -1, -1) if reverse else range(4)
                    js = range(NHT - 1, -1, -1) if reverse else range(NHT)
                    for ng in ngs:
                        ps2 = [psA.tile([128, 512], f32, tag="acc", name=f"ps2_{b}_{ng}_{m2}")
                               for m2 in range(2)]
                        first = True
                        for j in js:
                            if (ng, j) in wt_cache:
                                wt = wt_cache.pop((ng, j))
                            else:
                                wt = pW2.tile([128, 512], bf16, tag="w2")
                                r0 = (ng * NHT + j) * 128
                                nc.sync.dma_start(out=wt[:], in_=WQ2[r0:r0 + 128, :])
                                if not reverse and (ng == 3 or (ng == 2 and j >= 10)):
                                    wt_cache[(ng, j)] = wt
                            for m2 in range(2):
                                nc.tensor.matmul(
                                    ps2[m2][:],
                                    xq2T[b][j][:, m2 * 128:(m2 + 1) * 128],
                                    wt[:], start=first, stop=(j == (0 if reverse else NHT - 1)))
                            first = False
                        for m2 in range(2):
                            ot = pO.tile([128, 512], f32, tag="ot")
                            nc.scalar.mul(ot[:], ps2[m2][:], d2col[b][m2][:])
                            r0 = (b * 2 + m2) * 128
                            for ch in range(2):
                                nc.sync.dma_start(
                                    out=OUT[r0:r0 + 128,
                                            ng * 512 + ch * 256:ng * 512 + (ch + 1) * 256],
                                    in_=ot[:, ch * 256:(ch + 1) * 256])

                def fire_a2a2(b):
                    nc.gpsimd.collective_compute(
                        "AllToAll", ALU.bypass, replica_groups=[list(range(C))],
                        ins=[a2i[b][:].opt()], outs=[a2o[b][:].opt()])

                attn_unit(0, 0)
                attn_unit(0, 1)
                fire_a2a2(0)
                load_x2t(0, nc.sync)
                attn_unit(1, 0)
                d_quant(0)
                attn_unit(1, 1)
                fire_a2a2(1)
                load_x2t(1, nc.gpsimd)
                d_mm(0, False)
                d_quant(1, reverse=True)
                d_mm(1, True)

    nc.compile()
    return nc


_NC_CACHE = {}


def _get_nc():
    if "nc" not in _NC_CACHE:
        _NC_CACHE["nc"] = build_nc()
    return _NC_CACHE["nc"]


def _stripe(wt, nchunk):
    """[H, O] -> [(O//512)*16*128, 512] contiguous (ng, j)-stripe layout."""
    Hh, O = wt.shape
    a = wt.reshape(NHT, 128, O // 512, 512)          # [j, h, ng, c]
    a = a.transpose(2, 0, 1, 3)                      # [ng, j, h, c]
    return np.ascontiguousarray(a.reshape(-1, 512))


def kernel(x, w_norm, w_qkv, w_out):
    x = np.asarray(x, dtype=np.float32)
    w_norm = np.asarray(w_norm, dtype=np.float32)
    w_qkv = np.asarray(w_qkv, dtype=np.float32)
    w_out = np.asarray(w_out, dtype=np.float32)

    def tern(w):
        ws = np.float32(1.0) / np.clip(np.mean(np.abs(w)), np.float32(1e-5), None).astype(np.float32)
        wq = np.clip(np.round(w * ws), -1.0, 1.0).astype(np.float32)
        return wq, (np.float32(1.0) / ws).astype(np.float32)

    wq1, s_w1 = tern(w_qkv)
    wq2, s_w2 = tern(w_out)
    # permute q head blocks: new col hp*1024 + dest*128 + (h%2)*64 + d
    hperm = np.empty(NH, np.int64)
    for h in range(NH):
        hperm[(h % 4) // 2 * 16 + (h // 4) * 2 + (h % 2)] = h
    qperm = (hperm[:, None] * HD + np.arange(HD)[None, :]).reshape(-1)
    wq1p = wq1.copy()
    wq1p[:NH * HD] = wq1[qperm]
    wq1t = _stripe(np.ascontiguousarray(wq1p.T), QKV_O // 512).astype(ml_dtypes.bfloat16)
    wq2t = _stripe(np.ascontiguousarray(wq2.T), H // 512).astype(ml_dtypes.bfloat16)

    inv_freq = (1.0 / THETA ** (np.arange(0, HD, 2, dtype=np.float32) / HD)).astype(np.float32)
    t_pos = np.arange(S, dtype=np.float32)
    freqs = t_pos[:, None] * inv_freq[None, :]
    cos_full = np.cos(freqs).astype(np.float32)
    sin_full = np.sin(freqs).astype(np.float32)

    trimask = np.triu(np.ones((128, 128), np.float32)).astype(ml_dtypes.bfloat16)
    sw1 = np.array([[s_w1 / np.float32(127.0)]], dtype=np.float32)
    sw2 = np.array([[s_w2 / np.float32(127.0)]], dtype=np.float32)
    wn2d = w_norm.reshape(1, H)

    in_maps = []
    for i in range(C):
        xc = np.ascontiguousarray(
            np.concatenate([x[0, i * SC:(i + 1) * SC, :], x[1, i * SC:(i + 1) * SC, :]], axis=0))
        in_maps.append({
            "x": xc,
            "wn": wn2d,
            "wq1t": wq1t,
            "wq2t": wq2t,
            "cosb": np.ascontiguousarray(np.tile(cos_full[i * SC:(i + 1) * SC, :], (1, 8))),
            "sinb": np.ascontiguousarray(np.tile(sin_full[i * SC:(i + 1) * SC, :], (1, 8))),
            "trimask": trimask,
            "sw1": sw1,
            "sw2": sw2,
        })

    nc = _get_nc()
    res = bass_utils.run_bass_kernel_spmd(nc, in_maps, core_ids=list(range(C)))

    out = np.empty((B, S, H), dtype=np.float32)
    for i in range(C):
        ci = res.results[i]["out"]
        for b in range(B):
            out[b, i * SC:(i + 1) * SC, :] = ci[b * SC:(b + 1) * SC, :]
    return out


# revision 59
# speedup vs baseline: 1.0165x; 1.0048x over previous
"""Distributed Trainium2 Bass kernel for BitNet-style attention block.

Sharding: sequence-parallel projections + (batch x kv-head) parallel attention,
stitched with per-batch AllToAll collectives (split so comm overlaps compute).

Per core (core i):
  A. RMSNorm + per-token absmax quantization of its 512-token chunk.
  B. qkv projection as exact integer bf16 matmul against host-prequantized
     ternary weights, dequant, RoPE on q/k, scatter into per-batch A2A bufs.
  C. AllToAll #1 (k/v/qA/qB fired as their data completes) -> core i holds
     full-sequence q/k/v for kv-head i of each batch; causal attention with
     transposed scores, exp on ACT, and a TRANSPOSED attention*V (stationary
     [v|ones], moving probabilities, 512-wide) that emits out^T [d, qi] plus
     rowsums; normalization via gpsimd partition-broadcast of 1/rowsum.
  D. AllToAll #2 ships out^T (hidden-major rows) -> core i holds x^T
     [2048 hidden, 256 tok] per batch; per-token quantization via gpsimd
     partition all-reduce absmax, integer matmul with ternary output weights
     (no PE transposes needed - x^T rows are already the contract dim).
"""
import sys
sys.path.insert(0, "/opt/trn_rl_repo")
import numpy as np
import ml_dtypes
import concourse.bass as bass
import concourse.tile as tile
from concourse import bacc, mybir
from concourse import bass_utils
from concourse import bass_isa
from concourse.masks import make_identity

f32 = mybir.dt.float32
bf16 = mybir.dt.bfloat16
f8 = mybir.dt.float8e4
FT = mybir.ActivationFunctionType
ALU = mybir.AluOpType

B, S, H = 2, 2048, 2048
NH, NKV, HD = 32, 8, 64
G = NH // NKV                    # 4
QKV_O = (NH + 2 * NKV) * HD      # 3072
EPS = 1e-5
THETA = 10000.0
C = 8
SC = S // C                      # 256 positions per core
TOK = B * SC                     # 512 token rows per core
MAGIC = float(1.5 * 2.0 ** 23)   # RNE integer rounding for |v| < 2^22
NT = TOK // 128                  # 4 token tiles
NHT = H // 128                   # 16 h-tiles
NKT = S // 128                   # 16 kj tiles

# a2a1 split into four column-group collectives fired as their data completes:
#   k: per-batch [k 64] -> [8, 128, 128]  (dup built locally via 2 transpose loads)
#   v: per-batch [v 64] -> [8, 128, 128]
#   qA/qB: per-batch head-pair [2 heads = 128] -> [8, 256, 256] each
# q heads are permuted host-side: col hp*1024 + dest*128 + hh*64 + d
# a2a2 is transposed: rows = dest*256 + head*64 + d, cols = dest's 256 tokens.


def _dap(t_ap, extra, dims):
    return bass.AP(tensor=t_ap.tensor, offset=t_ap.offset + extra, ap=[list(d) for d in dims])


def build_nc():
    nc = bacc.Bacc("TRN2", target_bir_lowering=False, debug=False, num_devices=C)

    x_in = nc.dram_tensor("x", [TOK, H], f32, kind="ExternalInput")
    wn_in = nc.dram_tensor("wn", [1, H], f32, kind="ExternalInput")
    # contiguous stripes: row ((ng*16+j)*128 + h_local), 512 cols each
    wq1t_in = nc.dram_tensor("wq1t", [(QKV_O // 512) * NHT * 128, 512], bf16, kind="ExternalInput")
    wq2t_in = nc.dram_tensor("wq2t", [(H // 512) * NHT * 128, 512], bf16, kind="ExternalInput")
    cos_in = nc.dram_tensor("cosb", [SC, 8 * 32], f32, kind="ExternalInput")
    sin_in = nc.dram_tensor("sinb", [SC, 8 * 32], f32, kind="ExternalInput")
    tri_in = nc.dram_tensor("trimask", [128, 128], bf16, kind="ExternalInput")
    sw1_in = nc.dram_tensor("sw1", [1, 1], f32, kind="ExternalInput")
    sw2_in = nc.dram_tensor("sw2", [1, 1], f32, kind="ExternalInput")
    out_ext = nc.dram_tensor("out", [TOK, H], f32, kind="ExternalOutput")

    X = x_in.ap()
    WQ1 = wq1t_in.ap()
    WQ2 = wq2t_in.ap()
    OUT = out_ext.ap()

    with tile.TileContext(nc) as tc:
        from contextlib import ExitStack
        with ExitStack() as top:
            dram = top.enter_context(tc.tile_pool(name="dram", bufs=1, space="DRAM"))
            const = top.enter_context(tc.tile_pool(name="const", bufs=1))
            smalls = top.enter_context(tc.tile_pool(name="smalls", bufs=1))
            psA = top.enter_context(tc.tile_pool(name="psA", bufs=4, space="PSUM"))
            psS = top.enter_context(tc.tile_pool(name="psS", bufs=2, space="PSUM"))

            # ---------------- DRAM scratch ----------------
            aq_i = [dram.tile([C * SC, 256], bf16, name=f"aq_i{hp}") for hp in range(2)]
            aq_o = [dram.tile([C * SC, 256], bf16, name=f"aq_o{hp}") for hp in range(2)]
            ak_i = dram.tile([C * SC, 128], bf16, name="ak_i")
            ak_o = dram.tile([C * SC, 128], bf16, name="ak_o")
            av_i = dram.tile([C * SC, 128], bf16, name="av_i")
            av_o = dram.tile([C * SC, 128], bf16, name="av_o")
            a2i = [dram.tile([C * SC, G * HD], bf16, name=f"a2i_{b}") for b in range(B)]
            a2o = [dram.tile([C * SC, G * HD], bf16, name=f"a2o_{b}") for b in range(B)]
            d2dr = dram.tile([B, SC], f32, name="d2dr")

            # ---------------- constants ----------------
            wnorm_b = const.tile([128, H], f32)
            wnr = const.tile([1, H], f32)
            nc.sync.dma_start(out=wnr[0:1, :], in_=wn_in.ap()[0:1, :])
            nc.gpsimd.partition_broadcast(wnorm_b[:, :], wnr[0:1, :], channels=128)
            trim = const.tile([128, 128], bf16)
            nc.sync.dma_start(out=trim[:], in_=tri_in.ap()[:, :])
            sw1b = const.tile([128, 1], f32)
            nc.sync.dma_start(out=sw1b[:], in_=_dap(sw1_in.ap(), 0, [[0, 128], [1, 1]]))
            sw2b = const.tile([128, 1], f32)
            nc.sync.dma_start(out=sw2b[:], in_=_dap(sw2_in.ap(), 0, [[0, 128], [1, 1]]))
            epsb = const.tile([128, 1], f32)
            nc.vector.memset(epsb[:], EPS)
            magicb = const.tile([128, 1], f32)
            nc.vector.memset(magicb[:], MAGIC)
            ident = const.tile([128, 128], bf16)
            make_identity(nc, ident[:])

            d1s = [smalls.tile([128, 1], f32, name=f"d1_{m}") for m in range(NT)]

            xqT_pool = top.enter_context(tc.tile_pool(name="xqT", bufs=NHT))
            pQT = top.enter_context(tc.tile_pool(name="pQT", bufs=4))
            pKT = top.enter_context(tc.tile_pool(name="pKT", bufs=3))
            pVA = top.enter_context(tc.tile_pool(name="pVA", bufs=2 * NKT))

            # ================= Stage A: RMSNorm + quantize =================
            with ExitStack() as sa:
                pA = sa.enter_context(tc.tile_pool(name="pA", bufs=2))
                pXQ = sa.enter_context(tc.tile_pool(name="pXQ", bufs=NT))
                pSc = sa.enter_context(tc.tile_pool(name="pASc", bufs=4))
                xqms = []
                for m in range(NT):
                    xa = pA.tile([128, H], f32, tag="xa")
                    for xc in range(8):
                        nc.sync.dma_start(out=xa[:, xc * 256:(xc + 1) * 256],
                                          in_=X[m * 128:(m + 1) * 128, xc * 256:(xc + 1) * 256])
                    sq = pA.tile([128, H], f32, tag="sq")
                    ssq = pSc.tile([128, 1], f32, tag="ssq")
                    nc.scalar.activation(out=sq[:], in_=xa[:], func=FT.Square, accum_out=ssq[:])
                    xw = pA.tile([128, H], f32, tag="xw")
                    nc.vector.tensor_tensor(xw[:], xa[:], wnorm_b[:], ALU.mult)
                    std = pSc.tile([128, 1], f32, tag="std")
                    nc.scalar.activation(out=std[:], in_=ssq[:], func=FT.Sqrt,
                                         bias=epsb[:], scale=1.0 / H)
                    rstd = pSc.tile([128, 1], f32, tag="rstd")
                    nc.vector.reciprocal(rstd[:], std[:])
                    mx = pSc.tile([128, 1], f32, tag="mx")
                    nc.vector.tensor_reduce(mx[:], xw[:], mybir.AxisListType.X, ALU.max,
                                            apply_absolute_value=True)
                    mp = pSc.tile([128, 1], f32, tag="mp")
                    nc.vector.tensor_scalar(mp[:], mx[:], rstd[:], 1e-5, ALU.mult, ALU.max)
                    nc.vector.tensor_tensor(d1s[m][:], mp[:], sw1b[:], ALU.mult)
                    rmp = pSc.tile([128, 1], f32, tag="rmp")
                    nc.vector.reciprocal(rmp[:], mp[:])
                    csc = pSc.tile([128, 1], f32, tag="csc")
                    nc.vector.tensor_scalar(csc[:], rmp[:], rstd[:], 127.0, ALU.mult, ALU.mult)
                    t1 = pA.tile([128, H], f32, tag="t1")
                    nc.scalar.activation(out=t1[:], in_=xw[:], func=FT.Identity,
                                         scale=csc[:], bias=magicb[:])
                    xqm = pXQ.tile([128, H], bf16, tag="xqm", name=f"xqm_{m}")
                    nc.vector.tensor_scalar(xqm[:], t1[:], MAGIC, None, ALU.subtract)
                    xqms.append(xqm)

                # transposed activations via PE (keeps the DMA queues free)
                xqT = []
                for j in range(NHT):
                    t = xqT_pool.tile([128, TOK], bf16, name=f"xqT_{j}", tag="xqT")
                    xqT.append(t)
                for m in range(NT):
                    for j in range(NHT):
                        tp = psS.tile([128, 128], bf16, tag="st", name=f"tp_{m}_{j}")
                        nc.tensor.transpose(tp[:], xqms[m][:, j * 128:(j + 1) * 128], ident[:])
                        nc.vector.tensor_copy(xqT[j][:, m * 128:(m + 1) * 128], tp[:])

            # ================= Stage B: qkv matmul + RoPE + scatter ========
            with ExitStack() as sb:
                pW = sb.enter_context(tc.tile_pool(name="pW", bufs=28))
                pQC = sb.enter_context(tc.tile_pool(name="pQC", bufs=7))
                pRT = sb.enter_context(tc.tile_pool(name="pRT", bufs=3))
                pSend = sb.enter_context(tc.tile_pool(name="pSend", bufs=NT))
                pCos = sb.enter_context(tc.tile_pool(name="pCos", bufs=1))

                cosr = []
                sinr = []
                for par in range(2):
                    ct = pCos.tile([128, 8 * 32], f32, name=f"cosr_{par}")
                    nc.sync.dma_start(out=ct[:], in_=cos_in.ap()[par * 128:(par + 1) * 128, :])
                    st_ = pCos.tile([128, 8 * 32], f32, name=f"sinr_{par}")
                    nc.sync.dma_start(out=st_[:], in_=sin_in.ap()[par * 128:(par + 1) * 128, :])
                    cosr.append(ct)
                    sinr.append(st_)

                sends = [pSend.tile([128, QKV_O], bf16, name=f"sends_{m}", tag="sends")
                         for m in range(NT)]

                qTs = [[None] * 2 for _ in range(B)]
                KBs = [None] * B
                vas = [[None] * NKT for _ in range(B)]

                def load_kb(eng):
                    # KBoth rows 0:64 = k(b0), rows 64:128 = k(b1); per-batch
                    # dup tiles give each batch both PE row halves.
                    KBoth = pKT.tile([128, S], bf16, name="KBoth", tag="kT")
                    KB0d = pKT.tile([128, S], bf16, name="KB0d", tag="kT")
                    KB1d = pKT.tile([128, S], bf16, name="KB1d", tag="kT")
                    for c4 in range(4):
                        cs = slice(c4 * 512, (c4 + 1) * 512)
                        eng.dma_start(out=KBoth[:, cs],
                                      in_=ak_o[c4 * 512:(c4 + 1) * 512, :],
                                      transpose=True)
                        nc.gpsimd.dma_start(out=KB0d[64:128, cs], in_=KBoth[0:64, cs])
                        nc.gpsimd.dma_start(out=KB1d[0:64, cs], in_=KBoth[64:128, cs])
                    KBs[0] = (KBoth, KB0d)
                    KBs[1] = (KB1d, KBoth)

                def load_qt(hp, b, eng):
                    t = pQT.tile([128, S], bf16, name=f"qT_{b}_{hp}", tag="qT")
                    for c4 in range(4):
                        eng.dma_start(
                            out=t[:, c4 * 512:(c4 + 1) * 512],
                            in_=aq_o[hp][c4 * 512:(c4 + 1) * 512,
                                         b * 128:(b + 1) * 128],
                            transpose=True)
                    qTs[b][hp] = t

                def load_vas(eng):
                    for b in range(B):
                        for kt in range(NKT):
                            t = pVA.tile([128, 65], bf16, name=f"va_{b}_{kt}", tag="va")
                            eng.dma_start(
                                out=t[:, 0:64],
                                in_=av_o[kt * 128:(kt + 1) * 128, b * 64:(b + 1) * 64])
                            nc.vector.memset(t[:, 64:65], 1.0)
                            vas[b][kt] = t

                for ng in (4, 0, 1, 5, 2, 3):
                    psq = [psA.tile([128, 512], f32, tag="acc", name=f"qkvp_{ng}_{m}")
                           for m in range(NT)]
                    for j in range(NHT):
                        wt = pW.tile([128, 512], bf16, tag="w1")
                        r0 = (ng * NHT + j) * 128
                        nc.sync.dma_start(out=wt[:], in_=WQ1[r0:r0 + 128, :])
                        for m in range(NT):
                            nc.tensor.matmul(psq[m][:], xqT[j][:, m * 128:(m + 1) * 128], wt[:],
                                             start=(j == 0), stop=(j == NHT - 1))
                    for m in range(NT):
                        par = m % 2
                        if ng < 5:
                            qc_t = pQC.tile([128, 512], f32, tag="qc")
                            nc.vector.tensor_scalar(qc_t[:], psq[m][:], d1s[m][:],
                                                    None, ALU.mult)
                            xv = qc_t[:].rearrange("p (h t d) -> p h t d", t=2, d=32)
                            xr = xv[:, :, 0, :]
                            xi = xv[:, :, 1, :]
                            cv = cosr[par][:].rearrange("p (h d) -> p h d", d=32)
                            sv = sinr[par][:].rearrange("p (h d) -> p h d", d=32)
                            ov = sends[m][:, ng * 512:(ng + 1) * 512].rearrange(
                                "p (h t d) -> p h t d", t=2, d=32)
                            o_r = ov[:, :, 0, :]
                            o_i = ov[:, :, 1, :]
                            ta = pRT.tile([128, 256], f32, tag="ta")
                            tb = pRT.tile([128, 256], f32, tag="tb")
                            tav = ta[:].rearrange("p (h d) -> p h d", d=32)
                            tbv = tb[:].rearrange("p (h d) -> p h d", d=32)
                            tc_ = pRT.tile([128, 256], f32, tag="tc")
                            td = pRT.tile([128, 256], f32, tag="td")
                            tcv = tc_[:].rearrange("p (h d) -> p h d", d=32)
                            tdv = td[:].rearrange("p (h d) -> p h d", d=32)
                            nc.vector.tensor_tensor(tav, xr, cv, ALU.mult)
                            nc.vector.tensor_tensor(tbv, xi, sv, ALU.mult)
                            nc.vector.tensor_tensor(o_r, tav, tbv, ALU.subtract)
                            nc.vector.tensor_tensor(tcv, xr, sv, ALU.mult)
                            nc.vector.tensor_tensor(tdv, xi, cv, ALU.mult)
                            nc.vector.tensor_tensor(o_i, tcv, tdv, ALU.add)
                        else:
                            nc.vector.tensor_scalar(sends[m][:, ng * 512:(ng + 1) * 512],
                                                    psq[m][:], d1s[m][:], None, ALU.mult)

                    if ng in (1, 3):
                        # a head-pair's q chunks complete: scatter + fire its AllToAll
                        hp = ng // 2
                        for m in range(NT):
                            b = m // 2
                            par = m % 2
                            base = par * 128 * 256 + b * 128
                            for dh in range(2):
                                nc.sync.dma_start(
                                    out=_dap(aq_i[hp][:], base + dh * 4 * SC * 256,
                                             [[256, 128], [SC * 256, 4], [1, 128]]),
                                    in_=sends[m][:, hp * 1024 + dh * 512:
                                                 hp * 1024 + (dh + 1) * 512].rearrange(
                                        "p (j c) -> p j c", j=4))
                        nc.gpsimd.collective_compute(
                            "AllToAll", ALU.bypass, replica_groups=[list(range(C))],
                            ins=[aq_i[hp][:].opt()], outs=[aq_o[hp][:].opt()])
                        if ng == 1:
                            # ACT is idle until the first exp: issue the
                            # hp0/b0 q transpose-load right behind its a2a
                            load_qt(0, 0, nc.scalar)
                    elif ng == 4:
                        # k chunk complete: scatter + k AllToAll (dup built locally)
                        for m in range(NT):
                            b = m // 2
                            par = m % 2
                            base = par * 128 * 128 + b * 64
                            nc.sync.dma_start(
                                out=_dap(ak_i[:], base,
                                         [[128, 128], [SC * 128, 8], [1, 64]]),
                                in_=sends[m][:, 2048:2560].rearrange(
                                    "p (j c) -> p j c", j=8))
                        nc.gpsimd.collective_compute(
                            "AllToAll", ALU.bypass, replica_groups=[list(range(C))],
                            ins=[ak_i[:].opt()], outs=[ak_o[:].opt()])
                    elif ng == 5:
                        # v chunk complete: scatter + v AllToAll
                        for m in range(NT):
                            b = m // 2
                            par = m % 2
                            base = par * 128 * 128 + b * 64
                            nc.sync.dma_start(
                                out=_dap(av_i[:], base,
                                         [[128, 128], [SC * 128, 8], [1, 64]]),
                                in_=sends[m][:, 2560:3072].rearrange("p (j c) -> p j c", j=8))
                        nc.gpsimd.collective_compute(
                            "AllToAll", ALU.bypass, replica_groups=[list(range(C))],
                            ins=[av_i[:].opt()], outs=[av_o[:].opt()])

                # all attention-side loads issue from Sync after the stripe
                # issues; a2i scatters live on gpsimd so a blocking load here
                # stalls nothing
                load_kb(nc.sync)
                load_vas(nc.sync)
                load_qt(1, 0, nc.sync)
                load_qt(0, 1, nc.sync)
                load_qt(1, 1, nc.sync)

            # ================= Stage C: attention =========================
            # and Stage D: out projection, interleaved per-batch.
            with ExitStack() as sc:
                pEX = sc.enter_context(tc.tile_pool(name="pEX", bufs=28))
                pOB = sc.enter_context(tc.tile_pool(name="pOB", bufs=4))
                pR = sc.enter_context(tc.tile_pool(name="pR", bufs=3))
                pD = sc.enter_context(tc.tile_pool(name="pD", bufs=3))
                pXT2 = sc.enter_context(tc.tile_pool(name="pXT2", bufs=NHT + 2))
                pW2 = sc.enter_context(tc.tile_pool(name="pW2", bufs=24))
                pO = sc.enter_context(tc.tile_pool(name="pO", bufs=3))
                pDs = sc.enter_context(tc.tile_pool(name="pDs", bufs=2))

                def attn_unit(b, hp):
                    """Causal attention for batch b, q-head-pair hp.
                    AV matmuls of qc-1 are interleaved between QK tiles of qc
                    at ~2:1 so the exp stream always has a fresh score tile and
                    the PE never drains during an AV block."""
                    KH0, KH1 = KBs[b]
                    vab = vas[b]
                    qTx = qTs[b][hp]

                    def av_gen(qc, exs):
                        for h in range(2):
                            hg = hp * 2 + h
                            poT = psA.tile([128, 512], f32, tag="acc",
                                           name=f"poT_{b}_{hp}_{qc}_{h}")
                            last = 4 * qc + 3
                            for kt in range(last + 1):
                                dpos = max(0, kt * 128 - qc * 512)
                                nc.tensor.matmul(
                                    poT[0:65, dpos:512],
                                    vab[kt][:, 0:65],
                                    exs[kt][:, h * 512 + dpos:(h + 1) * 512],
                                    start=(kt == 0), stop=(kt == last),
                                    skip_group_check=(kt > 0))
                                yield
                            rs = pR.tile([1, 512], f32, tag="rs")
                            nc.vector.tensor_copy(rs[0:1, :], poT[64:65, 0:512])
                            rq = pR.tile([1, 512], f32, tag="rq")
                            nc.vector.reciprocal_approx_fast(out=rq[0:1, :],
                                                             in_=rs[0:1, :])
                            bc = pR.tile([64, 512], f32, tag="bc")
                            nc.gpsimd.partition_broadcast(bc[:, :], rq[0:1, :], channels=64)
                            nrm = pOB.tile([64, 512], bf16, tag="nrm",
                                           name=f"nrm_{b}_{hp}_{qc}_{h}")
                            nc.vector.tensor_tensor(nrm[:], poT[0:64, 0:512], bc[:], ALU.mult)
                            for par in range(2):
                                nc.gpsimd.dma_start(
                                    out=_dap(a2i[b][:],
                                             (((2 * qc + par) * 256 + hg * 64) * 256),
                                             [[256, 64], [1, 256]]),
                                    in_=nrm[:, par * 256:(par + 1) * 256])
                            yield

                    gen = None
                    for qc in (3, 2, 1, 0):
                        exs = []
                        for kt in range(4 * qc + 4):
                            dpos = max(0, kt * 128 - qc * 512)
                            st = psS.tile([128, 1024], f32, tag="st",
                                          name=f"st_{b}_{hp}_{qc}_{kt}")
                            nc.tensor.matmul(
                                st[:, dpos:512],
                                KH0[0:64, kt * 128:(kt + 1) * 128],
                                qTx[0:64, qc * 512 + dpos:(qc + 1) * 512],
                                start=True, stop=True)
                            nc.tensor.matmul(
                                st[:, 512 + dpos:1024],
                                KH1[64:128, kt * 128:(kt + 1) * 128],
                                qTx[64:128, qc * 512 + dpos:(qc + 1) * 512],
                                start=True, stop=True, tile_position=(64, 0))
                            ex = pEX.tile([128, 1024], bf16, tag="ex",
                                          name=f"ex_{b}_{hp}_{qc}_{kt}")
                            stv = st[:].rearrange("p (h q) -> p h q", h=2)[:, :, dpos:512]
                            exv = ex[:].rearrange("p (h q) -> p h q", h=2)[:, :, dpos:512]
                            nc.scalar.activation(out=exv, in_=stv, func=FT.Exp, scale=0.125)
                            if kt >= 4 * qc:
                                for h in range(2):
                                    sl = ex[:, h * 512 + dpos:h * 512 + dpos + 128]
                                    nc.vector.tensor_tensor(sl, sl, trim[:], ALU.mult)
                            exs.append(ex)
                            if gen is not None:
                                next(gen, None)
                                next(gen, None)
                        if gen is not None:
                            for _ in gen:
                                pass
                        gen = av_gen(qc, exs)
                    for _ in gen:
                        pass

                xq2T = [[None] * NHT for _ in range(B)]
                d2col = [[None] * 2 for _ in range(B)]

                x2tt = {}

                def load_x2t(b, eng):
                    x2ts = []
                    for j in range(NHT):
                        xt = pXT2.tile([128, SC], bf16, tag="x2t", name=f"x2t_{b}_{j}")
                        eng.dma_start(out=xt[:], in_=a2o[b][j * 128:(j + 1) * 128, :])
                        x2ts.append(xt)
                    x2tt[b] = x2ts

                def d_quant(b, reverse=False):
                    """Per-token absmax + int8 quantization of x^T for batch b.
                    Produces xq2T tiles in the same j order d_mm consumes."""
                    x2ts = x2tt[b]
                    jorder = list(range(NHT - 1, -1, -1)) if reverse else list(range(NHT))
                    # elementwise |.| max-accumulate on DVE, then one cross-partition
                    # absmax on gpsimd
                    acc = pDs.tile([128, SC], bf16, tag="aacc", name=f"aacc_{b}", bufs=2)
                    nc.vector.scalar_tensor_tensor(acc[:], x2ts[0][:], -1.0, x2ts[0][:],
                                                   ALU.mult, ALU.max)
                    for j in range(1, NHT):
                        nc.vector.scalar_tensor_tensor(acc[:], x2ts[j][:], -1.0, acc[:],
                                                       ALU.mult, ALU.max)
                        nc.vector.tensor_tensor(acc[:], acc[:], x2ts[j][:], ALU.max)
                    pm = pDs.tile([128, SC], f32, tag="pm", bufs=2)
                    nc.gpsimd.partition_all_reduce(pm[:], acc[:], 128,
                                                   bass_isa.ReduceOp.absmax)
                    mp = pDs.tile([1, SC], f32, tag="mprow", name=f"mprow_{b}", bufs=1)
                    nc.vector.tensor_scalar(mp[0:1, :], pm[0:1, :], 1e-5, None, ALU.max)
                    # d2 row -> DRAM -> read back as a column (per 128-token tile)
                    d2r = pDs.tile([1, SC], f32, tag="d2row", name=f"d2row_{b}", bufs=1)
                    nc.vector.tensor_scalar(d2r[0:1, :], mp[0:1, :],
                                            sw2b[0:1, 0:1], None, ALU.mult)
                    nc.sync.dma_start(out=d2dr[b:b + 1, :], in_=d2r[0:1, :])
                    for m2 in range(2):
                        dc = pDs.tile([128, 1], f32, tag="d2c", name=f"d2c_{b}_{m2}", bufs=4)
                        nc.sync.dma_start(
                            out=dc[:],
                            in_=_dap(d2dr[:], b * SC + m2 * 128, [[1, 128], [SC * B, 1]]))
                        d2col[b][m2] = dc
                    # scale row 127/max -> broadcast to all partitions
                    scr = pDs.tile([1, SC], f32, tag="scrow", name=f"scrow_{b}", bufs=1)
                    nc.vector.reciprocal_approx_fast(out=scr[0:1, :], in_=mp[0:1, :])
                    sc2 = pDs.tile([1, SC], f32, tag="scrow2", name=f"scrow2_{b}", bufs=1)
                    nc.vector.tensor_scalar(sc2[0:1, :], scr[0:1, :], 127.0, None, ALU.mult)
                    scb = pDs.tile([128, SC], f32, tag="scb", name=f"scb_{b}")
                    nc.gpsimd.partition_broadcast(scb[:, :], sc2[0:1, :], channels=128)
                    tqs = {}
                    for j in jorder:
                        tq = pD.tile([128, SC], f32, tag="tq", bufs=3)
                        nc.vector.tensor_tensor(tq[:], x2ts[j][:], scb[:], ALU.mult)
                        tq2 = pD.tile([128, SC], f32, tag="tq2", bufs=3)
                        nc.scalar.add(tq2[:], tq[:], magicb[:])
                        tqs[j] = tq2
                    for j in jorder:
                        xqt = pXT2.tile([128, SC], bf16, tag="xq2t", name=f"xq2t_{b}_{j}")
                        nc.vector.tensor_scalar(xqt[:], tqs[j][:], MAGIC, None, ALU.subtract)
                        xq2T[b][j] = xqt

                wt_cache = {}

                def d_mm(b, reverse):
                    """Out projection matmuls + dequant + store for batch b.
                    Pass 2 runs in reverse stripe order and reuses the last
                    pW2-ring stripes of pass 1 that are still resident."""
                    ngs = range(3, -1, -1) if reverse else range(4)
                    js = range(NHT - 1, -1, -1) if reverse else range(NHT)
                    for ng in ngs:
                        ps2 = [psA.tile([128, 512], f32, tag="acc", name=f"ps2_{b}_{ng}_{m2}")
                               for m2 in range(2)]
                        first = True
                        for j in js:
                            if (ng, j) in wt_cache:
                                wt = wt_cache.pop((ng, j))
                            else:
                                wt = pW2.tile([128, 512], bf16, tag="w2")
                                r0 = (ng * NHT + j) * 128
                                nc.sync.dma_start(out=wt[:], in_=WQ2[r0:r0 + 128, :])
                                if not reverse and (ng == 3 or (ng == 2 and j >= 10)):
                                    wt_cache[(ng, j)] = wt
                            for m2 in range(2):
                                nc.tensor.matmul(
                                    ps2[m2][:],
                                    xq2T[b][j][:, m2 * 128:(m2 + 1) * 128],
                                    wt[:], start=first, stop=(j == (0 if reverse else NHT - 1)))
                            first = False
                        for m2 in range(2):
                            ot = pO.tile([128, 512], f32, tag="ot")
                            nc.scalar.mul(ot[:], ps2[m2][:], d2col[b][m2][:])
                            r0 = (b * 2 + m2) * 128
                            for ch in range(2):
                                nc.sync.dma_start(
                                    out=OUT[r0:r0 + 128,
                                            ng * 512 + ch * 256:ng * 512 + (ch + 1) * 256],
                                    in_=ot[:, ch * 256:(ch + 1) * 256])

                def fire_a2a2(b):
                    nc.gpsimd.collective_compute(
                        "AllToAll", ALU.bypass, replica_groups=[list(range(C))],
                        ins=[a2i[b][:].opt()], outs=[a2o[b][:].opt()])

                attn_unit(0, 0)
                attn_unit(0, 1)
                fire_a2a2(0)
                load_x2t(0, nc.sync)
                attn_unit(1, 0)
                d_quant(0)
                attn_unit(1, 1)
                fire_a2a2(1)
                load_x2t(1, nc.gpsimd)
                d_mm(0, False)
                d_quant(1, reverse=True)
                d_mm(1, True)

    nc.compile()
    return nc


_NC_CACHE = {}


def _get_nc():
    if "nc" not in _NC_CACHE:
        _NC_CACHE["nc"] = build_nc()
    return _NC_CACHE["nc"]


def _stripe(wt, nchunk):
    """[H, O] -> [(O//512)*16*128, 512] contiguous (ng, j)-stripe layout."""
    Hh, O = wt.shape
    a = wt.reshape(NHT, 128, O // 512, 512)          # [j, h, ng, c]
    a = a.transpose(2, 0, 1, 3)                      # [ng, j, h, c]
    return np.ascontiguousarray(a.reshape(-1, 512))


def kernel(x, w_norm, w_qkv, w_out):
    x = np.asarray(x, dtype=np.float32)
    w_norm = np.asarray(w_norm, dtype=np.float32)
    w_qkv = np.asarray(w_qkv, dtype=np.float32)
    w_out = np.asarray(w_out, dtype=np.float32)

    def tern(w):
        ws = np.float32(1.0) / np.clip(np.mean(np.abs(w)), np.float32(1e-5), None).astype(np.float32)
        wq = np.clip(np.round(w * ws), -1.0, 1.0).astype(np.float32)
        return wq, (np.float32(1.0) / ws).astype(np.float32)

    wq1, s_w1 = tern(w_qkv)
    wq2, s_w2 = tern(w_out)
    # permute q head blocks: new col hp*1024 + dest*128 + (h%2)*64 + d
    hperm = np.empty(NH, np.int64)
    for h in range(NH):
        hperm[(h % 4) // 2 * 16 + (h // 4) * 2 + (h % 2)] = h
    qperm = (hperm[:, None] * HD + np.arange(HD)[None, :]).reshape(-1)
    wq1p = wq1.copy()
    wq1p[:NH * HD] = wq1[qperm]
    wq1t = _stripe(np.ascontiguousarray(wq1p.T), QKV_O // 512).astype(ml_dtypes.bfloat16)
    wq2t = _stripe(np.ascontiguousarray(wq2.T), H // 512).astype(ml_dtypes.bfloat16)

    inv_freq = (1.0 / THETA ** (np.arange(0, HD, 2, dtype=np.float32) / HD)).astype(np.float32)
    t_pos = np.arange(S, dtype=np.float32)
    freqs = t_pos[:, None] * inv_freq[None, :]
    cos_full = np.cos(freqs).astype(np.float32)
    sin_full = np.sin(freqs).astype(np.float32)

    trimask = np.triu(np.ones((128, 128), np.float32)).astype(ml_dtypes.bfloat16)
    sw1 = np.array([[s_w1 / np.float32(127.0)]], dtype=np.float32)
    sw2 = np.array([[s_w2 / np.float32(127.0)]], dtype=np.float32)
    wn2d = w_norm.reshape(1, H)

    in_maps = []
    for i in range(C):
        xc = np.ascontiguousarray(
            np.concatenate([x[0, i * SC:(i + 1) * SC, :], x[1, i * SC:(i + 1) * SC, :]], axis=0))
        in_maps.append({
            "x": xc,
            "wn": wn2d,
            "wq1t": wq1t,
            "wq2t": wq2t,
            "cosb": np.ascontiguousarray(np.tile(cos_full[i * SC:(i + 1) * SC, :], (1, 8))),
            "sinb": np.ascontiguousarray(np.tile(sin_full[i * SC:(i + 1) * SC, :], (1, 8))),
            "trimask": trimask,
            "sw1": sw1,
            "sw2": sw2,
        })

    nc = _get_nc()
    res = bass_utils.run_bass_kernel_spmd(nc, in_maps, core_ids=list(range(C)))

    out = np.empty((B, S, H), dtype=np.float32)
    for i in range(C):
        ci = res.results[i]["out"]
        for b in range(B):
            out[b, i * SC:(i + 1) * SC, :] = ci[b * SC:(b + 1) * SC, :]
    return out
